# revision 21
# baseline (speedup 1.0000x reference)
"""Trainium2 Bass kernel for nn_Attention_51307679318359.

Multi-head attention (B=2, S=2048, D=2048, H=16, HD=128) with RoPE and an
additive mask, sharded over 8 NeuronCores as (batch x head-group): each core
computes 1 batch and 4 heads (512 channels), producing a partial output that
the host sums over head-groups.

All matmul operands are bf16 (fast weight load, half the HBM traffic); PSUM
accumulation stays fp32, so accuracy is well inside the 2e-2 gate.  Q/K/V and
the attention output live entirely in SBUF -- no DRAM spills.

Per-core dataflow (activations transposed, channels on partitions):
  Warmup: ~3.5us of junk matmuls on a memset tile so the PE HAM clock-gate is
  released (2.4 GHz) by the time the first weight/x pieces land; the startup
  DMAs are split into fine pieces across 4 queues so the first real matmul
  issues ~7us in.
  Phase A: QT = wq' @ xT, KT = wk' @ xT with rotate-half permuted weights,
  RoPE applied on the PSUM output (ACT copy + DVE muls); V = xT' @ wv.
  Phase B+C (fused): per (sq-chunk, head), scoresT tiles are matmul'd in
  PAIRS into a 2-bank PSUM tile so one ACT exp covers both (halves the
  per-instruction exp overhead); multiplicative exp(mask) patterns only on
  the diagonal sub-blocks; AV + DVE/GpSimd-accumulated denominator; deferred
  normalization folded into the PSUM->SBUF evacuation (at_ps * bc broadcast).
  The out-projection (C) work of chunk jq-1 is interleaved MM-by-MM into
  chunk jq's score/AV stream, so the PE never idles on exp latency and the
  old phase-C window disappears entirely.
"""

import math

import numpy as np
import ml_dtypes

import concourse.bass as bass
import concourse.mybir as mybir
import concourse.tile as tile
from concourse import bacc
from concourse import bass_utils

F32 = mybir.dt.float32
F32R = mybir.dt.float32r
BF16 = mybir.dt.bfloat16
ADD = mybir.AluOpType.add
MULT = mybir.AluOpType.mult

B, S, D = 2, 2048, 2048
H, HD = 16, 128
NCORES = 8
GROUPS = NCORES // B          # 4 head-groups
HPG = H // GROUPS             # 4 heads per group
C = HPG * HD                  # 512 per-core channels
P = 128
CH = 512                      # phase-A s-chunk width
SQ = 512                      # phase-B sq-chunk width
NKT = D // P                  # 16 k-tiles
NKB = 4                       # k-blocks
NKI = NKT // NKB              # k-tiles per block
NCT = C // P                  # 4 column tiles (= heads per group)
SCALE = 1.0 / math.sqrt(HD)
NEG_THRESH = -1e8             # "masked out" threshold

N_WARM = 40                   # junk warmup matmuls (N=64)
SKEWP = 2                     # item lookahead between front and back stages
EVAC_DELAY = 3                # items after a head's last back before bc+evac
C_QUOTA = 3                   # max C-ops popped per item step
C_RESERVE = 16                 # C-ops held back as tail filler
FUSED_EVAC = True             # DVE TT with two PSUM sources (at_ps * bc_ps)
GP_PATTERNS = False            # pattern muls on GpSimd instead of DVE
GP_NARROW_ADDS = False         # narrow acc adds on GpSimd

_PROGRAM_CACHE = {}


def _pre_wqk(wT):
    """(d, c) -> [P, kb, ct, ki, 128] k-block-major contiguous bf16."""
    a = wT.reshape(NKB, NKI, P, NCT, P)              # [kb, ki, p, ct, col]
    return np.ascontiguousarray(a.transpose(2, 0, 3, 1, 4)).astype(
        ml_dtypes.bfloat16)


def _pre_w(wT):
    """(d, c) -> (128, d//128, c) partition-major contiguous bf16."""
    d, c = wT.shape
    return np.ascontiguousarray(
        wT.reshape(d // P, P, c).transpose(1, 0, 2)).astype(ml_dtypes.bfloat16)


def _pre_x(xT):
    """(d, s) -> (s//CH, 128, d//128, CH) chunk-major contiguous bf16."""
    d, s = xT.shape
    return np.ascontiguousarray(
        xT.reshape(d // P, P, s // CH, CH).transpose(2, 1, 0, 3)).astype(
            ml_dtypes.bfloat16)


def _classify_mask(mask):
    """Classify transposed-mask 128x128 sub-blocks per (sq-chunk jq, sk-tile i,
    sub-col m).  Returns (live, patterns):
      live[jq] = list of (i, c0, c1, [(m, pat_idx), ...]): per live sk tile,
        the live column range [c0, c1) within the chunk and the patterned
        128-col sub-blocks; the first entry always has c0 == 0 and c1 == SQ.
      patterns: (nblk, 128, 128) f32 holding exp(maskT sub-block).
    """
    maskT = np.ascontiguousarray(mask.T)
    s = mask.shape[0]
    n_j = s // SQ
    n_i = s // P
    n_m = SQ // P
    patterns = []
    pat_idx = {}

    def blk_of(jq, i, m):
        return maskT[i * P:(i + 1) * P, jq * SQ + m * P:jq * SQ + (m + 1) * P]

    def add_pattern(blk):
        key = blk.tobytes()
        if key not in pat_idx:
            pat_idx[key] = len(patterns)
            with np.errstate(over='ignore'):
                patterns.append(np.exp(blk.astype(np.float64)).astype(np.float32))
        return pat_idx[key]

    cls = {}
    for jq in range(n_j):
        for i in range(n_i):
            for m in range(n_m):
                blk = blk_of(jq, i, m)
                if np.all(blk == 0.0):
                    cls[(jq, i, m)] = 'plain'
                elif np.all(blk <= NEG_THRESH):
                    cls[(jq, i, m)] = 'skip'
                else:
                    cls[(jq, i, m)] = add_pattern(blk)

    def full_rows(jq):
        rows = []
        for i in range(n_i):
            pats = []
            for m in range(n_m):
                c = cls[(jq, i, m)]
                if c == 'skip':
                    pats.append((m, add_pattern(blk_of(jq, i, m))))
                elif isinstance(c, int):
                    pats.append((m, c))
            rows.append((i, 0, SQ, pats))
        return rows

    live = {}
    for jq in range(n_j):
        rows = []
        for i in range(n_i):
            ms = [m for m in range(n_m) if cls[(jq, i, m)] != 'skip']
            if not ms:
                continue
            m0, m1 = min(ms), max(ms)
            pats = []
            for m in range(m0, m1 + 1):
                c = cls[(jq, i, m)]
                if c == 'skip':
                    # interior hole: apply its (all-zero) pattern explicitly
                    pats.append((m, add_pattern(blk_of(jq, i, m))))
                elif isinstance(c, int):
                    pats.append((m, c))
            rows.append((i, m0 * P, (m1 + 1) * P, pats))
        # PSUM accumulation needs a full-width first tile, and softmax needs
        # every column covered; fall back to no narrowing for odd masks.
        cov = np.zeros(SQ, bool)
        for (_, c0, c1, _) in rows:
            cov[c0:c1] = True
        if not rows or not cov.all():
            rows = full_rows(jq)
        elif rows[0][1] != 0 or rows[0][2] != SQ:
            # move a full-width tile to the front if one exists, else fall back
            fw = [r for r in rows if r[1] == 0 and r[2] == SQ]
            if fw:
                rows.remove(fw[0])
                rows.insert(0, fw[0])
            else:
                rows = full_rows(jq)
        live[jq] = rows
    pats = np.stack(patterns, 0).astype(np.float32) if patterns else \
        np.zeros((1, P, P), np.float32)
    return live, pats


def _live_key(live):
    return tuple(sorted(
        (jq, tuple((i, c0, c1, tuple(p)) for (i, c0, c1, p) in rows))
        for jq, rows in live.items()))


def _make_items(rows):
    """Group a head-chunk's rows into exp-batchable items.

    Row 0 (full width) stays single: its exp writes the denominator
    accumulator tile directly.  Remaining rows are paired so one ACT exp
    covers a 2-bank PSUM tile; a trailing odd row stays single."""
    items = [[rows[0]]]
    k = 1
    while k < len(rows):
        if k + 1 < len(rows):
            items.append([rows[k], rows[k + 1]])
            k += 2
        else:
            items.append([rows[k]])
            k += 1
    return items


def _build(live, nblk, s=S, d=D):
    """Build + compile the per-core SPMD program."""
    n_j = s // SQ
    n_i = s // P
    n_ch = s // CH
    nd = d // SQ

    nc = bacc.Bacc("TRN2", target_bir_lowering=False, debug=False)
    xT = nc.dram_tensor("xT", (n_ch, P, NKT, CH), BF16, kind="ExternalInput")
    wqT = nc.dram_tensor("wqT", (P, NKB, NCT, NKI, P), BF16,
                         kind="ExternalInput")
    wkT = nc.dram_tensor("wkT", (P, NKB, NCT, NKI, P), BF16,
                         kind="ExternalInput")
    wvT = nc.dram_tensor("wvT", (P, NKT, C), BF16, kind="ExternalInput")
    woT = nc.dram_tensor("woT", (P, HPG, d), BF16, kind="ExternalInput")
    cosP = nc.dram_tensor("cosP", (HD, s), BF16, kind="ExternalInput")
    sinSw = nc.dram_tensor("sinSw", (HD, s), BF16, kind="ExternalInput")
    mblk = nc.dram_tensor("mblk", (P, nblk, P), BF16, kind="ExternalInput")
    onesb = nc.dram_tensor("onesb", (P, 1), BF16, kind="ExternalInput")
    onesf = nc.dram_tensor("onesf", (1, P), F32, kind="ExternalInput")
    out = nc.dram_tensor("out", (s, d), BF16, kind="ExternalOutput")

    with tile.TileContext(nc) as tc:
        with tc.tile_pool(name="const", bufs=1) as const:
            ones_b = const.tile([P, 1], BF16)
            ones_f = const.tile([1, P], F32R)
            mblk_t = const.tile([P, nblk, P], BF16)

            qh_sb = const.tile([P, HPG, s], BF16)
            kh_sb = const.tile([P, HPG, s], BF16)
            v_sb = const.tile([P, n_i, C], BF16)
            attn_t = const.tile([P, HPG, s], BF16)
            wo_t = const.tile([P, HPG, d], BF16)

            # ---------------- Phase A: QKV projections + RoPE ----------------
            with tc.tile_pool(name="wres", bufs=1) as wres, \
                 tc.tile_pool(name="xc", bufs=3) as xcp, \
                 tc.tile_pool(name="trig", bufs=1) as trig, \
                 tc.tile_pool(name="ptmp", bufs=2) as ptmp, \
                 tc.tile_pool(name="psA", bufs=6, space="PSUM") as psA, \
                 tc.tile_pool(name="psV", bufs=2, space="PSUM") as psV:
                wq_t = wres.tile([P, NKB, NCT, NKI, P], BF16)
                wk_t = wres.tile([P, NKB, NCT, NKI, P], BF16)
                wv_t = wres.tile([P, NKT, C], BF16)
                junk = wres.tile([P, 64], BF16)
                dum1 = wres.tile([1, P], F32)
                dum2 = wres.tile([P, P], F32)
                cos_t = trig.tile([P, s], BF16)
                sin_t = trig.tile([P, s], BF16)

                # PE warmup: junk matmuls on a memset tile release the HAM
                # clock gate (1.2 -> 2.4 GHz) before the first real matmul
                nc.gpsimd.memset(junk[:], 0)
                nc.gpsimd.memset(dum1[:], 1)
                psW = psA.tile([P, CH], F32, tag="ps_qk", name="ps_warm")
                for _ in range(N_WARM):
                    nc.tensor.matmul(psW[0:64, 0:64], junk[:], junk[:],
                                     start=True, stop=True)

                # startup (3 DMA queues: sync, scalar, gpsimd).  The first
                # matmul group needs only the wq[kb0,ct0] piece and xc0
                # k-tile 0, so sync/scalar lead with 128KB pieces in need
                # order; wk + trig ride gpsimd (K starts ~14us after Q).
                for kb in range(NKB):
                    for ct in range(NCT):
                        nc.sync.dma_start(wq_t[:, kb, ct], wqT[:, kb, ct])
                xc0 = xcp.tile([P, NKT, CH], BF16, tag="xc")
                for k in range(NKT):
                    nc.scalar.dma_start(xc0[:, k:k + 1, :], xT[0, :, k:k + 1, :])
                nc.gpsimd.dma_start(cos_t[:], cosP[:])
                nc.gpsimd.dma_start(sin_t[:], sinSw[:])
                nc.gpsimd.dma_start(mblk_t[:], mblk[:])
                nc.gpsimd.dma_start(ones_b[:], onesb[:])
                nc.gpsimd.dma_start(ones_f[:], onesf[:].bitcast(F32R))
                for ct in range(NCT):
                    nc.gpsimd.dma_start(wk_t[:, 0, ct], wkT[:, 0, ct])
                for kb in range(1, NKB):
                    nc.gpsimd.dma_start(wk_t[:, kb], wkT[:, kb])
                nc.gpsimd.dma_start(wo_t[:], woT[:])
                # prime the GpSimd custom-op program (PartitionBroadcast's
                # first use pays a ~7us Q7 library load; do it under phase A)
                nc.gpsimd.partition_broadcast(dum2[:], dum1[:])

                def rope_store(ps, dst, sl):
                    # rotate-half RoPE on a finished PSUM tile -> bf16 SBUF
                    pc = ptmp.tile([P, CH], F32, tag="pc")
                    nc.scalar.activation(
                        pc[:], ps[:], mybir.ActivationFunctionType.Copy)
                    t1 = ptmp.tile([P, CH], F32, tag="t1")
                    nc.vector.tensor_tensor(t1[:], pc[:], cos_t[:, sl], MULT)
                    t2 = ptmp.tile([P, CH], F32, tag="t2")
                    nc.vector.tensor_tensor(
                        t2[0:64, :], pc[64:128, :], sin_t[64:128, sl], MULT)
                    nc.vector.tensor_tensor(
                        t2[64:128, :], pc[0:64, :], sin_t[0:64, sl], MULT)
                    nc.vector.tensor_tensor(dst, t1[:], t2[:], ADD)

                def v_proj(jv, xcv):
                    for st2 in range(CH // P):
                        st = (jv * CH) // P + st2
                        psv = psV.tile([P, C], F32, tag="ps_v")
                        for k in range(NKT):
                            nc.tensor.matmul(
                                psv[:], xcv[:, k, st2 * P:(st2 + 1) * P],
                                wv_t[:, k, :],
                                start=(k == 0), stop=(k == NKT - 1))
                        # ACT copy keeps the DVE queue clear for phase B
                        nc.scalar.activation(
                            v_sb[:, st, :], psv[:],
                            mybir.ActivationFunctionType.Copy)

                prev = None
                nxt = None
                for j in range(n_ch):
                    sl = slice(j * CH, (j + 1) * CH)
                    xc = xc0 if j == 0 else nxt
                    # prefetch the next chunk a full iteration ahead, split
                    # across the scalar and sync queues so neither starves
                    if j + 1 < n_ch:
                        nxt = xcp.tile([P, NKT, CH], BF16, tag="xc")
                        for kb in range(NKB):
                            eng = nc.scalar if kb % 2 == 0 else nc.sync
                            eng.dma_start(
                                nxt[:, kb * NKI:(kb + 1) * NKI, :],
                                xT[j + 1, :, kb * NKI:(kb + 1) * NKI, :])
                    if j == 0:
                        # wv rides behind chunk 1's prefetch on both queues
                        nc.scalar.dma_start(wv_t[:, 0:NKT // 2, :],
                                            wvT[:, 0:NKT // 2, :])
                        nc.sync.dma_start(wv_t[:, NKT // 2:, :],
                                          wvT[:, NKT // 2:, :])
                    for (wt, dst) in ((wq_t, qh_sb), (wk_t, kh_sb)):
                        if j == 0:
                            # kb-major: consumes startup DMA pieces in
                            # arrival order
                            pss = [psA.tile([P, CH], F32, tag="ps_qk",
                                            name=f"ps_qk{ct}")
                                   for ct in range(NCT)]
                            for kb in range(NKB):
                                for ct in range(NCT):
                                    for ki in range(NKI):
                                        nc.tensor.matmul(
                                            pss[ct][:], wt[:, kb, ct, ki, :],
                                            xc[:, kb * NKI + ki, :],
                                            start=(kb == 0 and ki == 0),
                                            stop=(kb == NKB - 1 and
                                                  ki == NKI - 1),
                                            skip_group_check=True)
                            for ct in range(NCT):
                                rope_store(pss[ct], dst[:, ct, sl], sl)
                        else:
                            # ct-major: each rope starts right after its own
                            # 16-MM chain, so the DVE queue drains during the
                            # chunk instead of piling up at the A->B boundary
                            for ct in range(NCT):
                                psc = psA.tile([P, CH], F32, tag="ps_qk",
                                               name=f"ps_qk{ct}")
                                for kb in range(NKB):
                                    for ki in range(NKI):
                                        nc.tensor.matmul(
                                            psc[:], wt[:, kb, ct, ki, :],
                                            xc[:, kb * NKI + ki, :],
                                            start=(kb == 0 and ki == 0),
                                            stop=(kb == NKB - 1 and
                                                  ki == NKI - 1),
                                            skip_group_check=True)
                                rope_store(psc, dst[:, ct, sl], sl)
                    if prev is not None:
                        v_proj(j - 1, prev)
                    prev = xc
                v_proj(n_ch - 1, prev)

            # ---------------- Phase B+C fused ----------------
            # Global software pipeline over (chunk jq, head h, item): scores
            # matmuls + pair-batched exp run SKEWP items ahead of the AV /
            # denominator stage, and the out-projection of chunk jq-1 is
            # drip-fed into the stream so the PE never waits on ACT.
            entries = []
            head_last = {}
            for jq in range(n_j):
                rows = live[jq]
                items = _make_items(rows)
                for h in range(HPG):
                    n0 = 0
                    for k2, it in enumerate(items):
                        entries.append(dict(
                            jq=jq, h=h, item=it, n0=n0,
                            first=(k2 == 0), last=(k2 == len(items) - 1),
                            r=len(rows)))
                        n0 += len(it)
                    head_last[(jq, h)] = len(entries) - 1

            evac_at = {}
            for (jq, h), last in head_last.items():
                evac_at.setdefault(last + SKEWP + EVAC_DELAY + h, []).append(
                    (jq, h))
            c_release = {head_last[(jq, HPG - 1)] + SKEWP + EVAC_DELAY + 2: jq
                         for jq in range(n_j - 1)}

            with tc.tile_pool(name="rcp", bufs=3) as rcp, \
                 tc.tile_pool(name="sm", bufs=6) as smp, \
                 tc.tile_pool(name="pr", bufs=SKEWP + 3) as prp, \
                 tc.tile_pool(name="acc", bufs=8) as accp, \
                 tc.tile_pool(name="og", bufs=2) as ogp, \
                 tc.tile_pool(name="psS", bufs=2, space="PSUM") as psS, \
                 tc.tile_pool(name="psAt", bufs=2, space="PSUM") as psAt, \
                 tc.tile_pool(name="psC", bufs=2, space="PSUM") as psC:

                state = {}    # (jq,h) -> dict(at_ps, acc)
                rcs = {}
                c_queue = []
                c_po = {}     # current po tile for the active C dch chain
                c_og = {}     # st -> og tile

                def pat_mul(dst_ap, pidx):
                    if GP_PATTERNS:
                        nc.gpsimd.scalar_tensor_tensor(
                            dst_ap, dst_ap, 1.0, mblk_t[:, pidx, :],
                            MULT, MULT)
                    else:
                        nc.vector.tensor_tensor(
                            dst_ap, dst_ap, mblk_t[:, pidx, :], MULT)

                def emit_front(e):
                    jq, h, it = e['jq'], e['h'], e['item']
                    qb = jq * SQ
                    if e['first']:
                        state[(jq, h)] = dict(
                            acc=accp.tile([P, SQ], BF16, tag="acc", name="acc"))
                    st_ = state[(jq, h)]
                    if e['first']:
                        # row 0: single full-width exp writes the denominator
                        # accumulator directly
                        (i, c0, c1, pats) = it[0]
                        sc = psS.tile([P, 2 * SQ], F32, tag="sc", name="sc")
                        nc.tensor.matmul(
                            sc[:, c0:c1],
                            kh_sb[:, h, i * P:(i + 1) * P],
                            qh_sb[:, h, qb + c0:qb + c1],
                            start=True, stop=True)
                        acc = st_['acc']
                        nc.scalar.activation(
                            acc[:, c0:c1], sc[:, c0:c1],
                            mybir.ActivationFunctionType.Exp, scale=SCALE)
                        for (m, pidx) in pats:
                            pat_mul(acc[:, m * P:(m + 1) * P], pidx)
                        e['src'] = acc
                        e['offs'] = [0]
                        return
                    sc = psS.tile([P, 2 * SQ], F32, tag="sc", name="sc")
                    pr = prp.tile([P, 2 * SQ], BF16, tag="pr", name="pr")
                    for k2, (i, c0, c1, pats) in enumerate(it):
                        off = k2 * SQ
                        nc.tensor.matmul(
                            sc[:, off + c0:off + c1],
                            kh_sb[:, h, i * P:(i + 1) * P],
                            qh_sb[:, h, qb + c0:qb + c1],
                            start=True, stop=True)
                    c0a = it[0][1]
                    end = (len(it) - 1) * SQ + it[-1][2]
                    nc.scalar.activation(
                        pr[:, c0a:end], sc[:, c0a:end],
                        mybir.ActivationFunctionType.Exp, scale=SCALE)
                    for k2, (i, c0, c1, pats) in enumerate(it):
                        off = k2 * SQ
                        for (m, pidx) in pats:
                            pat_mul(pr[:, off + m * P:off + (m + 1) * P], pidx)
                    e['src'] = pr
                    e['offs'] = [k2 * SQ for k2 in range(len(it))]

                def emit_back(e):
                    jq, h, it = e['jq'], e['h'], e['item']
                    st_ = state[(jq, h)]
                    src = e['src']
                    acc = st_['acc']
                    for k2, (i, c0, c1, pats) in enumerate(it):
                        n = e['n0'] + k2
                        off = e['offs'][k2]
                        if n == 0:
                            st_['at_ps'] = psAt.tile([P, SQ], F32, tag="at", name="at")
                        at_ps = st_['at_ps']
                        nc.tensor.matmul(
                            at_ps[:, c0:c1],
                            v_sb[:, i, h * HD:(h + 1) * HD],
                            src[:, off + c0:off + c1],
                            start=(n == 0), stop=(n == e['r'] - 1),
                            skip_group_check=True)
                        if n > 0:
                            wide = (c1 - c0) > 256
                            if wide or not GP_NARROW_ADDS:
                                nc.vector.tensor_tensor(
                                    acc[:, c0:c1], acc[:, c0:c1],
                                    src[:, off + c0:off + c1], ADD)
                            else:
                                nc.gpsimd.scalar_tensor_tensor(
                                    acc[:, c0:c1], acc[:, c0:c1], 1.0,
                                    src[:, off + c0:off + c1], MULT, ADD)

                def emit_dn(jq, h):
                    acc = state[(jq, h)]['acc']
                    dn_ps = psS.tile([P, 2 * SQ], F32, tag="sc", name="dn_ps")
                    nc.tensor.matmul(dn_ps[0:1, 0:SQ], ones_b[:], acc[:],
                                     start=True, stop=True)
                    dn_sb = smp.tile([1, SQ], F32, tag="dnsb", name="dn_sb")
                    last = (jq == n_j - 1 and h == HPG - 1)
                    if last:
                        nc.vector.tensor_copy(dn_sb[:], dn_ps[0:1, 0:SQ])
                    else:
                        nc.scalar.activation(dn_sb[:], dn_ps[0:1, 0:SQ],
                                             mybir.ActivationFunctionType.Copy)
                    if last:
                        # tail chain gates the final out-projection: skip
                        # the fold hops, reciprocal directly on (1,512)
                        rc1 = smp.tile([1, SQ], F32, tag="rc1", name="rc1d")
                        nc.vector.reciprocal_approx_fast(rc1[:], dn_sb[:])
                    else:
                        # reciprocal cost scales with free-size per lane:
                        # fold the (1,512) row to (4,128) via DMA first
                        dn4 = smp.tile([SQ // P, P], F32, tag="dn4",
                                       name="dn4")
                        nc.gpsimd.dma_start(dn4[:], dn_sb[:])
                        rc4 = smp.tile([SQ // P, P], F32, tag="rc4",
                                       name="rc4")
                        nc.vector.reciprocal_approx_fast(rc4[:], dn4[:])
                        rc1 = smp.tile([1, SQ], F32, tag="rc1", name="rc1")
                        nc.gpsimd.dma_start(rc1[:], rc4[:])
                    # broadcast 1/dn to all partitions on GpSimd so the DVE
                    # evacuation TT has a legal SBUF second operand and the
                    # PE skips the ones-broadcast matmul
                    bc_sb = rcp.tile([P, SQ], F32, tag="rc", name="bc_sb")
                    nc.gpsimd.partition_broadcast(bc_sb[:], rc1[:])
                    rcs[(jq, h)] = bc_sb
                    if jq == 0:
                        # chunk-0 heads are only ~2us apart: evacuate the AV
                        # accumulator unnormalized NOW so psAt never starves;
                        # the normalization TT happens at the usual evac slot
                        at_ps = state[(jq, h)].pop('at_ps')
                        at_sb = accp.tile([P, SQ], BF16, tag="acc",
                                          name="at_sb")
                        nc.vector.tensor_copy(at_sb[:], at_ps[:])
                        state[(jq, h)]['at_sb'] = at_sb

                def emit_evac(jq, h):
                    bc_sb = rcs.pop((jq, h))
                    qsl = slice(jq * SQ, (jq + 1) * SQ)
                    if jq == 0:
                        at_sb = state[(jq, h)].pop('at_sb')
                        nc.vector.tensor_tensor(
                            attn_t[:, h, qsl], at_sb[:], bc_sb[:], MULT)
                    else:
                        at_ps = state[(jq, h)].pop('at_ps')
                        nc.vector.tensor_tensor(
                            attn_t[:, h, qsl], at_ps[:], bc_sb[:], MULT)

                def push_c_ops(jq, final=False):
                    for st2 in range(SQ // P):
                        st = jq * (SQ // P) + st2
                        for dch in range(nd):
                            for ct in range(HPG):
                                c_queue.append((st, dch, ct, final))

                def pop_c(quota):
                    npop = min(quota, len(c_queue))
                    for _ in range(npop):
                        (st, dch, ct, final) = c_queue.pop(0)
                        if ct == 0 and dch == 0:
                            c_og[st] = ogp.tile([P, d], BF16, tag="og", name="og")
                        if ct == 0:
                            c_po[st] = psC.tile([P, SQ], F32, tag="po", name="po")
                        po = c_po[st]
                        nc.tensor.matmul(
                            po[:], attn_t[:, ct, st * P:(st + 1) * P],
                            wo_t[:, ct, dch * SQ:(dch + 1) * SQ],
                            start=(ct == 0), stop=(ct == HPG - 1))
                        if ct != HPG - 1:
                            continue
                        og = c_og[st]
                        osl = slice(dch * SQ, (dch + 1) * SQ)
                        if dch % 4 == 3:
                            nc.scalar.activation(
                                og[:, osl], po[:],
                                mybir.ActivationFunctionType.Copy)
                        else:
                            nc.vector.tensor_copy(og[:, osl], po[:])
                        rsl = slice(st * P, (st + 1) * P)
                        if final:
                            # final chunk: write each dch immediately on a
                            # rotating queue so the tail DMA drains early
                            eng = (nc.sync, nc.gpsimd, nc.scalar,
                                   nc.sync)[(st + dch) % 4]
                            eng.dma_start(out[rsl, osl], og[:, osl])
                        elif dch % 2 == 1:
                            eng = nc.sync if (st + dch) % 4 == 1 else nc.gpsimd
                            eng.dma_start(
                                out[rsl, (dch - 1) * SQ:(dch + 1) * SQ],
                                og[:, (dch - 1) * SQ:(dch + 1) * SQ])

                total = len(entries) + SKEWP
                for idx in range(total):
                    if idx < len(entries):
                        emit_front(entries[idx])
                    bidx = idx - SKEWP
                    if bidx >= 0:
                        e = entries[bidx]
                        emit_back(e)
                        if e['last']:
                            emit_dn(e['jq'], e['h'])
                    for (jq, h) in evac_at.get(idx, []):
                        emit_evac(jq, h)
                    if idx in c_release:
                        push_c_ops(c_release[idx])
                    if len(c_queue) > C_RESERVE:
                        pop_c(min(C_QUOTA, len(c_queue) - C_RESERVE))

                # tail: held-back C ops cover the last rc chain, then the
                # final chunk's evacs and out-projection drain
                pop_c(C_RESERVE)
                for k in sorted(k for k in evac_at if k >= total):
                    for (jq, h) in evac_at[k]:
                        emit_evac(jq, h)
                push_c_ops(n_j - 1, final=True)
                pop_c(len(c_queue))

    nc.compile()
    return nc


def _prep_host(inputs):
    """Shard + transpose the full inputs into 8 per-core input maps."""
    x = np.asarray(inputs["x"], np.float32)
    wq = np.asarray(inputs["wq"], np.float32)
    wk = np.asarray(inputs["wk"], np.float32)
    wv = np.asarray(inputs["wv"], np.float32)
    wo = np.asarray(inputs["wo"], np.float32)
    cos = np.asarray(inputs["cos"], np.float32)
    sin = np.asarray(inputs["sin"], np.float32)
    mask = np.asarray(inputs["mask"], np.float32)
    start_p = int(inputs["start_p"])

    s = x.shape[1]
    cos_u = cos[start_p:start_p + s]          # (s, HD/2)
    sin_u = sin[start_p:start_p + s]

    # rotate-half channel permutation within each head: [evens, odds]
    perm = np.concatenate(
        [h * HD + np.concatenate([np.arange(0, HD, 2), np.arange(1, HD, 2)])
         for h in range(H)])

    cosP = np.ascontiguousarray(
        np.concatenate([cos_u.T, cos_u.T], axis=0)).astype(
            ml_dtypes.bfloat16)                              # (128, s)
    sinSw = np.ascontiguousarray(
        np.concatenate([sin_u.T, -sin_u.T], axis=0)).astype(
            ml_dtypes.bfloat16)                              # (128, s)

    live, pats = _classify_mask(mask)
    onesb = np.ones((P, 1), ml_dtypes.bfloat16)
    onesf = np.ones((1, P), np.float32)
    mblk = np.ascontiguousarray(pats.transpose(1, 0, 2)).astype(
        ml_dtypes.bfloat16)

    in_maps = []
    for b in range(B):
        xTp = _pre_x(np.ascontiguousarray(x[b].T))
        for g in range(GROUPS):
            rows = perm[g * C:(g + 1) * C]
            in_maps.append({
                "xT": xTp,
                "wqT": _pre_wqk(wq[rows, :].T),
                "wkT": _pre_wqk(wk[rows, :].T),
                "wvT": _pre_w(wv[g * C:(g + 1) * C, :].T),
                "woT": _pre_w(wo[:, g * C:(g + 1) * C].T),
                "cosP": cosP,
                "sinSw": sinSw,
                "mblk": mblk,
                "onesb": onesb,
                "onesf": onesf,
            })
    return in_maps, live, pats


def _run(inputs, trace=False):
    in_maps, live, pats = _prep_host(inputs)
    key = (pats.shape[0], _live_key(live))
    if key not in _PROGRAM_CACHE:
        _PROGRAM_CACHE[key] = _build(live, pats.shape[0])
    nc = _PROGRAM_CACHE[key]
    res = bass_utils.run_bass_kernel_spmd(
        nc, in_maps, core_ids=list(range(NCORES)), trace=trace)
    out = np.zeros((B, S, D), np.float32)
    for b in range(B):
        acc = res.results[b * GROUPS]["out"].astype(np.float32)
        for g in range(1, GROUPS):
            acc += res.results[b * GROUPS + g]["out"].astype(np.float32)
        out[b] = acc
    return out, res


def kernel(**inputs):
    out, _ = _run(inputs, trace=False)
    return out


# revision 22
# speedup vs baseline: 1.0320x; 1.0320x over previous
"""Trainium2 Bass kernel for nn_Attention_51307679318359.

Multi-head attention (B=2, S=2048, D=2048, H=16, HD=128) with RoPE and an
additive mask, sharded over 8 NeuronCores as (batch x head-group): each core
computes 1 batch and 4 heads (512 channels), producing a partial output that
the host sums over head-groups.

All matmul operands are bf16 (fast weight load, half the HBM traffic); PSUM
accumulation stays fp32, so accuracy is well inside the 2e-2 gate.  Q/K/V and
the attention output live entirely in SBUF -- no DRAM spills.

Per-core dataflow (activations transposed, channels on partitions):
  Warmup: ~3.5us of junk matmuls on a memset tile so the PE HAM clock-gate is
  released (2.4 GHz) by the time the first weight/x pieces land; the startup
  DMAs are split into fine pieces across 4 queues so the first real matmul
  issues ~7us in.
  Phase A: QT = wq' @ xT, KT = wk' @ xT with rotate-half permuted weights,
  RoPE applied on the PSUM output (ACT copy + DVE muls); V = xT' @ wv.
  Phase B+C (fused): per (sq-chunk, head), scoresT tiles are matmul'd in
  PAIRS into a 2-bank PSUM tile so one ACT exp covers both (halves the
  per-instruction exp overhead); multiplicative exp(mask) patterns only on
  the diagonal sub-blocks; AV + DVE/GpSimd-accumulated denominator; deferred
  normalization folded into the PSUM->SBUF evacuation (at_ps * bc broadcast).
  The out-projection (C) work of chunk jq-1 is interleaved MM-by-MM into
  chunk jq's score/AV stream, so the PE never idles on exp latency and the
  old phase-C window disappears entirely.
"""

import math

import numpy as np
import ml_dtypes

import concourse.bass as bass
import concourse.mybir as mybir
import concourse.tile as tile
from concourse import bacc
from concourse import bass_utils

F32 = mybir.dt.float32
F32R = mybir.dt.float32r
BF16 = mybir.dt.bfloat16
ADD = mybir.AluOpType.add
MULT = mybir.AluOpType.mult

B, S, D = 2, 2048, 2048
H, HD = 16, 128
NCORES = 8
GROUPS = NCORES // B          # 4 head-groups
HPG = H // GROUPS             # 4 heads per group
C = HPG * HD                  # 512 per-core channels
P = 128
CH = 512                      # phase-A s-chunk width
SQ = 512                      # phase-B sq-chunk width
NKT = D // P                  # 16 k-tiles
NKB = 4                       # k-blocks
NKI = NKT // NKB              # k-tiles per block
NCT = C // P                  # 4 column tiles (= heads per group)
SCALE = 1.0 / math.sqrt(HD)
NEG_THRESH = -1e8             # "masked out" threshold

N_WARM = 40                   # junk warmup matmuls (N=64)
SKEWP = 2                     # item lookahead between front and back stages
EVAC_DELAY = 3                # items after a head's last back before bc+evac
C_QUOTA = 3                   # max C-ops popped per item step
C_RESERVE = 16                 # C-ops held back as tail filler
FUSED_EVAC = True             # DVE TT with two PSUM sources (at_ps * bc_ps)
GP_PATTERNS = False            # pattern muls on GpSimd instead of DVE
GP_NARROW_ADDS = False         # narrow acc adds on GpSimd

_PROGRAM_CACHE = {}


def _pre_wqk(wT):
    """(d, c) -> [P, kb, ct, ki, 128] k-block-major contiguous bf16."""
    a = wT.reshape(NKB, NKI, P, NCT, P)              # [kb, ki, p, ct, col]
    return np.ascontiguousarray(a.transpose(2, 0, 3, 1, 4)).astype(
        ml_dtypes.bfloat16)


def _pre_w(wT):
    """(d, c) -> (128, d//128, c) partition-major contiguous bf16."""
    d, c = wT.shape
    return np.ascontiguousarray(
        wT.reshape(d // P, P, c).transpose(1, 0, 2)).astype(ml_dtypes.bfloat16)


def _pre_x(xT):
    """(d, s) -> (s//CH, 128, d//128, CH) chunk-major contiguous bf16."""
    d, s = xT.shape
    return np.ascontiguousarray(
        xT.reshape(d // P, P, s // CH, CH).transpose(2, 1, 0, 3)).astype(
            ml_dtypes.bfloat16)


def _classify_mask(mask):
    """Classify transposed-mask 128x128 sub-blocks per (sq-chunk jq, sk-tile i,
    sub-col m).  Returns (live, patterns):
      live[jq] = list of (i, c0, c1, [(m, pat_idx), ...]): per live sk tile,
        the live column range [c0, c1) within the chunk and the patterned
        128-col sub-blocks; the first entry always has c0 == 0 and c1 == SQ.
      patterns: (nblk, 128, 128) f32 holding exp(maskT sub-block).
    """
    maskT = np.ascontiguousarray(mask.T)
    s = mask.shape[0]
    n_j = s // SQ
    n_i = s // P
    n_m = SQ // P
    patterns = []
    pat_idx = {}

    def blk_of(jq, i, m):
        return maskT[i * P:(i + 1) * P, jq * SQ + m * P:jq * SQ + (m + 1) * P]

    def add_pattern(blk):
        key = blk.tobytes()
        if key not in pat_idx:
            pat_idx[key] = len(patterns)
            with np.errstate(over='ignore'):
                patterns.append(np.exp(blk.astype(np.float64)).astype(np.float32))
        return pat_idx[key]

    cls = {}
    for jq in range(n_j):
        for i in range(n_i):
            for m in range(n_m):
                blk = blk_of(jq, i, m)
                if np.all(blk == 0.0):
                    cls[(jq, i, m)] = 'plain'
                elif np.all(blk <= NEG_THRESH):
                    cls[(jq, i, m)] = 'skip'
                else:
                    cls[(jq, i, m)] = add_pattern(blk)

    def full_rows(jq):
        rows = []
        for i in range(n_i):
            pats = []
            for m in range(n_m):
                c = cls[(jq, i, m)]
                if c == 'skip':
                    pats.append((m, add_pattern(blk_of(jq, i, m))))
                elif isinstance(c, int):
                    pats.append((m, c))
            rows.append((i, 0, SQ, pats))
        return rows

    live = {}
    for jq in range(n_j):
        rows = []
        for i in range(n_i):
            ms = [m for m in range(n_m) if cls[(jq, i, m)] != 'skip']
            if not ms:
                continue
            m0, m1 = min(ms), max(ms)
            pats = []
            for m in range(m0, m1 + 1):
                c = cls[(jq, i, m)]
                if c == 'skip':
                    # interior hole: apply its (all-zero) pattern explicitly
                    pats.append((m, add_pattern(blk_of(jq, i, m))))
                elif isinstance(c, int):
                    pats.append((m, c))
            rows.append((i, m0 * P, (m1 + 1) * P, pats))
        # PSUM accumulation needs a full-width first tile, and softmax needs
        # every column covered; fall back to no narrowing for odd masks.
        cov = np.zeros(SQ, bool)
        for (_, c0, c1, _) in rows:
            cov[c0:c1] = True
        if not rows or not cov.all():
            rows = full_rows(jq)
        elif rows[0][1] != 0 or rows[0][2] != SQ:
            # move a full-width tile to the front if one exists, else fall back
            fw = [r for r in rows if r[1] == 0 and r[2] == SQ]
            if fw:
                rows.remove(fw[0])
                rows.insert(0, fw[0])
            else:
                rows = full_rows(jq)
        live[jq] = rows
    pats = np.stack(patterns, 0).astype(np.float32) if patterns else \
        np.zeros((1, P, P), np.float32)
    return live, pats


def _live_key(live):
    return tuple(sorted(
        (jq, tuple((i, c0, c1, tuple(p)) for (i, c0, c1, p) in rows))
        for jq, rows in live.items()))


def _make_items(rows):
    """Group a head-chunk's rows into exp-batchable items.

    Row 0 (full width) stays single: its exp writes the denominator
    accumulator tile directly.  Remaining rows are paired so one ACT exp
    covers a 2-bank PSUM tile; a trailing odd row stays single."""
    items = [[rows[0]]]
    k = 1
    while k < len(rows):
        if k + 1 < len(rows):
            items.append([rows[k], rows[k + 1]])
            k += 2
        else:
            items.append([rows[k]])
            k += 1
    return items


def _build(live, nblk, s=S, d=D):
    """Build + compile the per-core SPMD program."""
    n_j = s // SQ
    n_i = s // P
    n_ch = s // CH
    nd = d // SQ

    nc = bacc.Bacc("TRN2", target_bir_lowering=False, debug=False)
    xT = nc.dram_tensor("xT", (n_ch, P, NKT, CH), BF16, kind="ExternalInput")
    wqT = nc.dram_tensor("wqT", (P, NKB, NCT, NKI, P), BF16,
                         kind="ExternalInput")
    wkT = nc.dram_tensor("wkT", (P, NKB, NCT, NKI, P), BF16,
                         kind="ExternalInput")
    wvT = nc.dram_tensor("wvT", (P, NKT, C), BF16, kind="ExternalInput")
    woT = nc.dram_tensor("woT", (P, HPG, d), BF16, kind="ExternalInput")
    cosP = nc.dram_tensor("cosP", (HD, s), BF16, kind="ExternalInput")
    sinSw = nc.dram_tensor("sinSw", (HD, s), BF16, kind="ExternalInput")
    mblk = nc.dram_tensor("mblk", (P, nblk, P), BF16, kind="ExternalInput")
    onesb = nc.dram_tensor("onesb", (P, 1), BF16, kind="ExternalInput")
    onesf = nc.dram_tensor("onesf", (1, P), F32, kind="ExternalInput")
    out = nc.dram_tensor("out", (s, d), BF16, kind="ExternalOutput")

    with tile.TileContext(nc) as tc:
        with tc.tile_pool(name="const", bufs=1) as const:
            ones_b = const.tile([P, 1], BF16)
            ones_f = const.tile([1, P], F32R)
            mblk_t = const.tile([P, nblk, P], BF16)

            qh_sb = const.tile([P, HPG, s], BF16)
            kh_sb = const.tile([P, HPG, s], BF16)
            v_sb = const.tile([P, n_i, C], BF16)
            attn_t = const.tile([P, HPG, s], BF16)
            wo_t = const.tile([P, HPG, d], BF16)

            # ---------------- Phase A: QKV projections + RoPE ----------------
            with tc.tile_pool(name="wres", bufs=1) as wres, \
                 tc.tile_pool(name="xc", bufs=3) as xcp, \
                 tc.tile_pool(name="trig", bufs=1) as trig, \
                 tc.tile_pool(name="ptmp", bufs=2) as ptmp, \
                 tc.tile_pool(name="psA", bufs=6, space="PSUM") as psA, \
                 tc.tile_pool(name="psV", bufs=2, space="PSUM") as psV:
                wq_t = wres.tile([P, NKB, NCT, NKI, P], BF16)
                wk_t = wres.tile([P, NKB, NCT, NKI, P], BF16)
                wv_t = wres.tile([P, NKT, C], BF16)
                junk = wres.tile([P, 64], BF16)
                dum1 = wres.tile([1, P], F32)
                dum2 = wres.tile([P, P], F32)
                cos_t = trig.tile([P, s], BF16)
                sin_t = trig.tile([P, s], BF16)

                # PE warmup: junk matmuls on a memset tile release the HAM
                # clock gate (1.2 -> 2.4 GHz) before the first real matmul
                nc.gpsimd.memset(junk[:], 0)
                nc.gpsimd.memset(dum1[:], 1)
                psW = psA.tile([P, CH], F32, tag="ps_qk", name="ps_warm")
                for _ in range(N_WARM):
                    nc.tensor.matmul(psW[0:64, 0:64], junk[:], junk[:],
                                     start=True, stop=True)

                # startup (3 DMA queues: sync, scalar, gpsimd).  The first
                # matmul group needs only the wq[kb0,ct0] piece and xc0
                # k-tile 0, so sync/scalar lead with 128KB pieces in need
                # order; wk + trig ride gpsimd (K starts ~14us after Q).
                for ct in range(NCT):
                    nc.sync.dma_start(wq_t[:, 0, ct], wqT[:, 0, ct])
                for kb in range(1, NKB):
                    nc.sync.dma_start(wq_t[:, kb], wqT[:, kb])
                xc0 = xcp.tile([P, NKT, CH], BF16, tag="xc")
                for k in range(NKI):
                    nc.scalar.dma_start(xc0[:, k:k + 1, :], xT[0, :, k:k + 1, :])
                for kb in range(1, NKB):
                    nc.scalar.dma_start(xc0[:, kb * NKI:(kb + 1) * NKI, :],
                                        xT[0, :, kb * NKI:(kb + 1) * NKI, :])
                nc.gpsimd.dma_start(cos_t[:], cosP[:])
                nc.gpsimd.dma_start(sin_t[:], sinSw[:])
                nc.gpsimd.dma_start(mblk_t[:], mblk[:])
                nc.gpsimd.dma_start(ones_b[:], onesb[:])
                nc.gpsimd.dma_start(ones_f[:], onesf[:].bitcast(F32R))
                for ct in range(NCT):
                    nc.gpsimd.dma_start(wk_t[:, 0, ct], wkT[:, 0, ct])
                for kb in range(1, NKB):
                    nc.gpsimd.dma_start(wk_t[:, kb], wkT[:, kb])
                nc.gpsimd.dma_start(wo_t[:], woT[:])
                # prime the GpSimd custom-op program (PartitionBroadcast's
                # first use pays a ~7us Q7 library load; do it under phase A)
                nc.gpsimd.partition_broadcast(dum2[:], dum1[:])

                def rope_store(ps, dst, sl):
                    # rotate-half RoPE on a finished PSUM tile -> bf16 SBUF
                    pc = ptmp.tile([P, CH], F32, tag="pc")
                    nc.scalar.activation(
                        pc[:], ps[:], mybir.ActivationFunctionType.Copy)
                    t1 = ptmp.tile([P, CH], F32, tag="t1")
                    nc.vector.tensor_tensor(t1[:], pc[:], cos_t[:, sl], MULT)
                    t2 = ptmp.tile([P, CH], F32, tag="t2")
                    nc.vector.tensor_tensor(
                        t2[0:64, :], pc[64:128, :], sin_t[64:128, sl], MULT)
                    nc.vector.tensor_tensor(
                        t2[64:128, :], pc[0:64, :], sin_t[0:64, sl], MULT)
                    nc.vector.tensor_tensor(dst, t1[:], t2[:], ADD)

                def v_proj(jv, xcv):
                    for st2 in range(CH // P):
                        st = (jv * CH) // P + st2
                        psv = psV.tile([P, C], F32, tag="ps_v")
                        for k in range(NKT):
                            nc.tensor.matmul(
                                psv[:], xcv[:, k, st2 * P:(st2 + 1) * P],
                                wv_t[:, k, :],
                                start=(k == 0), stop=(k == NKT - 1))
                        # ACT copy keeps the DVE queue clear for phase B
                        nc.scalar.activation(
                            v_sb[:, st, :], psv[:],
                            mybir.ActivationFunctionType.Copy)

                prev = None
                nxt = None
                for j in range(n_ch):
                    sl = slice(j * CH, (j + 1) * CH)
                    xc = xc0 if j == 0 else nxt
                    # prefetch the next chunk a full iteration ahead, split
                    # across the scalar and sync queues so neither starves
                    if j + 1 < n_ch:
                        nxt = xcp.tile([P, NKT, CH], BF16, tag="xc")
                        for kb in range(NKB):
                            eng = nc.scalar if kb % 2 == 0 else nc.sync
                            eng.dma_start(
                                nxt[:, kb * NKI:(kb + 1) * NKI, :],
                                xT[j + 1, :, kb * NKI:(kb + 1) * NKI, :])
                    if j == 0:
                        # wv rides behind chunk 1's prefetch on both queues
                        nc.scalar.dma_start(wv_t[:, 0:NKT // 2, :],
                                            wvT[:, 0:NKT // 2, :])
                        nc.sync.dma_start(wv_t[:, NKT // 2:, :],
                                          wvT[:, NKT // 2:, :])
                    for (wt, dst) in ((wq_t, qh_sb), (wk_t, kh_sb)):
                        if j == 0:
                            # kb-major: consumes startup DMA pieces in
                            # arrival order
                            pss = [psA.tile([P, CH], F32, tag="ps_qk",
                                            name=f"ps_qk{ct}")
                                   for ct in range(NCT)]
                            for kb in range(NKB):
                                for ct in range(NCT):
                                    for ki in range(NKI):
                                        nc.tensor.matmul(
                                            pss[ct][:], wt[:, kb, ct, ki, :],
                                            xc[:, kb * NKI + ki, :],
                                            start=(kb == 0 and ki == 0),
                                            stop=(kb == NKB - 1 and
                                                  ki == NKI - 1),
                                            skip_group_check=True)
                            for ct in range(NCT):
                                rope_store(pss[ct], dst[:, ct, sl], sl)
                        else:
                            # ct-major: each rope starts right after its own
                            # 16-MM chain, so the DVE queue drains during the
                            # chunk instead of piling up at the A->B boundary
                            for ct in range(NCT):
                                psc = psA.tile([P, CH], F32, tag="ps_qk",
                                               name=f"ps_qk{ct}")
                                for kb in range(NKB):
                                    for ki in range(NKI):
                                        nc.tensor.matmul(
                                            psc[:], wt[:, kb, ct, ki, :],
                                            xc[:, kb * NKI + ki, :],
                                            start=(kb == 0 and ki == 0),
                                            stop=(kb == NKB - 1 and
                                                  ki == NKI - 1),
                                            skip_group_check=True)
                                rope_store(psc, dst[:, ct, sl], sl)
                    if prev is not None:
                        v_proj(j - 1, prev)
                    prev = xc
                v_proj(n_ch - 1, prev)

            # ---------------- Phase B+C fused ----------------
            # Global software pipeline over (chunk jq, head h, item): scores
            # matmuls + pair-batched exp run SKEWP items ahead of the AV /
            # denominator stage, and the out-projection of chunk jq-1 is
            # drip-fed into the stream so the PE never waits on ACT.
            entries = []
            head_last = {}
            for jq in range(n_j):
                rows = live[jq]
                items = _make_items(rows)
                for h in range(HPG):
                    n0 = 0
                    for k2, it in enumerate(items):
                        entries.append(dict(
                            jq=jq, h=h, item=it, n0=n0,
                            first=(k2 == 0), last=(k2 == len(items) - 1),
                            r=len(rows)))
                        n0 += len(it)
                    head_last[(jq, h)] = len(entries) - 1

            evac_at = {}
            for (jq, h), last in head_last.items():
                evac_at.setdefault(last + SKEWP + EVAC_DELAY + h, []).append(
                    (jq, h))
            c_release = {head_last[(jq, HPG - 1)] + SKEWP + EVAC_DELAY + 2: jq
                         for jq in range(n_j - 1)}

            with tc.tile_pool(name="rcp", bufs=3) as rcp, \
                 tc.tile_pool(name="sm", bufs=6) as smp, \
                 tc.tile_pool(name="pr", bufs=SKEWP + 3) as prp, \
                 tc.tile_pool(name="acc", bufs=8) as accp, \
                 tc.tile_pool(name="og", bufs=2) as ogp, \
                 tc.tile_pool(name="psS", bufs=2, space="PSUM") as psS, \
                 tc.tile_pool(name="psAt", bufs=2, space="PSUM") as psAt, \
                 tc.tile_pool(name="psC", bufs=2, space="PSUM") as psC:

                state = {}    # (jq,h) -> dict(at_ps, acc)
                rcs = {}
                c_queue = []
                c_po = {}     # current po tile for the active C dch chain
                c_og = {}     # st -> og tile

                def pat_mul(dst_ap, pidx):
                    if GP_PATTERNS:
                        nc.gpsimd.scalar_tensor_tensor(
                            dst_ap, dst_ap, 1.0, mblk_t[:, pidx, :],
                            MULT, MULT)
                    else:
                        nc.vector.tensor_tensor(
                            dst_ap, dst_ap, mblk_t[:, pidx, :], MULT)

                def emit_front(e):
                    jq, h, it = e['jq'], e['h'], e['item']
                    qb = jq * SQ
                    if e['first']:
                        state[(jq, h)] = dict(
                            acc=accp.tile([P, SQ], BF16, tag="acc", name="acc"))
                    st_ = state[(jq, h)]
                    if e['first']:
                        # row 0: single full-width exp writes the denominator
                        # accumulator directly
                        (i, c0, c1, pats) = it[0]
                        sc = psS.tile([P, 2 * SQ], F32, tag="sc", name="sc")
                        nc.tensor.matmul(
                            sc[:, c0:c1],
                            kh_sb[:, h, i * P:(i + 1) * P],
                            qh_sb[:, h, qb + c0:qb + c1],
                            start=True, stop=True)
                        acc = st_['acc']
                        nc.scalar.activation(
                            acc[:, c0:c1], sc[:, c0:c1],
                            mybir.ActivationFunctionType.Exp, scale=SCALE)
                        for (m, pidx) in pats:
                            pat_mul(acc[:, m * P:(m + 1) * P], pidx)
                        e['src'] = acc
                        e['offs'] = [0]
                        return
                    sc = psS.tile([P, 2 * SQ], F32, tag="sc", name="sc")
                    pr = prp.tile([P, 2 * SQ], BF16, tag="pr", name="pr")
                    for k2, (i, c0, c1, pats) in enumerate(it):
                        off = k2 * SQ
                        nc.tensor.matmul(
                            sc[:, off + c0:off + c1],
                            kh_sb[:, h, i * P:(i + 1) * P],
                            qh_sb[:, h, qb + c0:qb + c1],
                            start=True, stop=True)
                    c0a = it[0][1]
                    end = (len(it) - 1) * SQ + it[-1][2]
                    nc.scalar.activation(
                        pr[:, c0a:end], sc[:, c0a:end],
                        mybir.ActivationFunctionType.Exp, scale=SCALE)
                    for k2, (i, c0, c1, pats) in enumerate(it):
                        off = k2 * SQ
                        for (m, pidx) in pats:
                            pat_mul(pr[:, off + m * P:off + (m + 1) * P], pidx)
                    e['src'] = pr
                    e['offs'] = [k2 * SQ for k2 in range(len(it))]

                def emit_back(e):
                    jq, h, it = e['jq'], e['h'], e['item']
                    st_ = state[(jq, h)]
                    src = e['src']
                    acc = st_['acc']
                    for k2, (i, c0, c1, pats) in enumerate(it):
                        n = e['n0'] + k2
                        off = e['offs'][k2]
                        if n == 0:
                            st_['at_ps'] = psAt.tile([P, SQ], F32, tag="at", name="at")
                        at_ps = st_['at_ps']
                        nc.tensor.matmul(
                            at_ps[:, c0:c1],
                            v_sb[:, i, h * HD:(h + 1) * HD],
                            src[:, off + c0:off + c1],
                            start=(n == 0), stop=(n == e['r'] - 1),
                            skip_group_check=True)
                        if n > 0:
                            wide = (c1 - c0) > 256
                            if wide or not GP_NARROW_ADDS:
                                nc.vector.tensor_tensor(
                                    acc[:, c0:c1], acc[:, c0:c1],
                                    src[:, off + c0:off + c1], ADD)
                            else:
                                nc.gpsimd.scalar_tensor_tensor(
                                    acc[:, c0:c1], acc[:, c0:c1], 1.0,
                                    src[:, off + c0:off + c1], MULT, ADD)

                def emit_dn(jq, h):
                    acc = state[(jq, h)]['acc']
                    dn_ps = psS.tile([P, 2 * SQ], F32, tag="sc", name="dn_ps")
                    nc.tensor.matmul(dn_ps[0:1, 0:SQ], ones_b[:], acc[:],
                                     start=True, stop=True)
                    dn_sb = smp.tile([1, SQ], F32, tag="dnsb", name="dn_sb")
                    last = (jq == n_j - 1 and h == HPG - 1)
                    if last:
                        nc.vector.tensor_copy(dn_sb[:], dn_ps[0:1, 0:SQ])
                    else:
                        nc.scalar.activation(dn_sb[:], dn_ps[0:1, 0:SQ],
                                             mybir.ActivationFunctionType.Copy)
                    if last:
                        # tail chain gates the final out-projection: skip
                        # the fold hops, reciprocal directly on (1,512)
                        rc1 = smp.tile([1, SQ], F32, tag="rc1", name="rc1d")
                        nc.vector.reciprocal_approx_fast(rc1[:], dn_sb[:])
                    else:
                        # reciprocal cost scales with free-size per lane:
                        # fold the (1,512) row to (4,128) via DMA first
                        dn4 = smp.tile([SQ // P, P], F32, tag="dn4",
                                       name="dn4")
                        nc.gpsimd.dma_start(dn4[:], dn_sb[:])
                        rc4 = smp.tile([SQ // P, P], F32, tag="rc4",
                                       name="rc4")
                        nc.vector.reciprocal_approx_fast(rc4[:], dn4[:])
                        rc1 = smp.tile([1, SQ], F32, tag="rc1", name="rc1")
                        nc.gpsimd.dma_start(rc1[:], rc4[:])
                    # broadcast 1/dn to all partitions on GpSimd so the DVE
                    # evacuation TT has a legal SBUF second operand and the
                    # PE skips the ones-broadcast matmul
                    bc_sb = rcp.tile([P, SQ], F32, tag="rc", name="bc_sb")
                    nc.gpsimd.partition_broadcast(bc_sb[:], rc1[:])
                    rcs[(jq, h)] = bc_sb
                    if jq == 0:
                        # chunk-0 heads are only ~2us apart: evacuate the AV
                        # accumulator unnormalized NOW so psAt never starves;
                        # the normalization TT happens at the usual evac slot
                        at_ps = state[(jq, h)].pop('at_ps')
                        at_sb = accp.tile([P, SQ], BF16, tag="acc",
                                          name="at_sb")
                        nc.vector.tensor_copy(at_sb[:], at_ps[:])
                        state[(jq, h)]['at_sb'] = at_sb

                def emit_evac(jq, h):
                    bc_sb = rcs.pop((jq, h))
                    qsl = slice(jq * SQ, (jq + 1) * SQ)
                    if jq == 0:
                        at_sb = state[(jq, h)].pop('at_sb')
                        nc.vector.tensor_tensor(
                            attn_t[:, h, qsl], at_sb[:], bc_sb[:], MULT)
                    else:
                        at_ps = state[(jq, h)].pop('at_ps')
                        nc.vector.tensor_tensor(
                            attn_t[:, h, qsl], at_ps[:], bc_sb[:], MULT)

                def push_c_ops(jq, final=False):
                    for st2 in range(SQ // P):
                        st = jq * (SQ // P) + st2
                        for dch in range(nd):
                            for ct in range(HPG):
                                c_queue.append((st, dch, ct, final))

                def pop_c(quota):
                    npop = min(quota, len(c_queue))
                    for _ in range(npop):
                        (st, dch, ct, final) = c_queue.pop(0)
                        if ct == 0 and dch == 0:
                            c_og[st] = ogp.tile([P, d], BF16, tag="og", name="og")
                        if ct == 0:
                            c_po[st] = psC.tile([P, SQ], F32, tag="po", name="po")
                        po = c_po[st]
                        nc.tensor.matmul(
                            po[:], attn_t[:, ct, st * P:(st + 1) * P],
                            wo_t[:, ct, dch * SQ:(dch + 1) * SQ],
                            start=(ct == 0), stop=(ct == HPG - 1))
                        if ct != HPG - 1:
                            continue
                        og = c_og[st]
                        osl = slice(dch * SQ, (dch + 1) * SQ)
                        if dch % 4 == 3:
                            nc.scalar.activation(
                                og[:, osl], po[:],
                                mybir.ActivationFunctionType.Copy)
                        else:
                            nc.vector.tensor_copy(og[:, osl], po[:])
                        rsl = slice(st * P, (st + 1) * P)
                        if final:
                            # final chunk: write each dch immediately on a
                            # rotating queue so the tail DMA drains early
                            eng = (nc.sync, nc.gpsimd, nc.scalar,
                                   nc.sync)[(st + dch) % 4]
                            eng.dma_start(out[rsl, osl], og[:, osl])
                        elif dch % 2 == 1:
                            eng = nc.sync if (st + dch) % 4 == 1 else nc.gpsimd
                            eng.dma_start(
                                out[rsl, (dch - 1) * SQ:(dch + 1) * SQ],
                                og[:, (dch - 1) * SQ:(dch + 1) * SQ])

                total = len(entries) + SKEWP
                for idx in range(total):
                    if idx < len(entries):
                        emit_front(entries[idx])
                    bidx = idx - SKEWP
                    if bidx >= 0:
                        e = entries[bidx]
                        emit_back(e)
                        if e['last']:
                            emit_dn(e['jq'], e['h'])
                    for (jq, h) in evac_at.get(idx, []):
                        emit_evac(jq, h)
                    if idx in c_release:
                        push_c_ops(c_release[idx])
                    if len(c_queue) > C_RESERVE:
                        pop_c(min(C_QUOTA, len(c_queue) - C_RESERVE))

                # tail: held-back C ops cover the last rc chain, then the
                # final chunk's evacs and out-projection drain
                pop_c(C_RESERVE)
                for k in sorted(k for k in evac_at if k >= total):
                    for (jq, h) in evac_at[k]:
                        emit_evac(jq, h)
                push_c_ops(n_j - 1, final=True)
                pop_c(len(c_queue))

    nc.compile()
    return nc


def _prep_host(inputs):
    """Shard + transpose the full inputs into 8 per-core input maps."""
    x = np.asarray(inputs["x"], np.float32)
    wq = np.asarray(inputs["wq"], np.float32)
    wk = np.asarray(inputs["wk"], np.float32)
    wv = np.asarray(inputs["wv"], np.float32)
    wo = np.asarray(inputs["wo"], np.float32)
    cos = np.asarray(inputs["cos"], np.float32)
    sin = np.asarray(inputs["sin"], np.float32)
    mask = np.asarray(inputs["mask"], np.float32)
    start_p = int(inputs["start_p"])

    s = x.shape[1]
    cos_u = cos[start_p:start_p + s]          # (s, HD/2)
    sin_u = sin[start_p:start_p + s]

    # rotate-half channel permutation within each head: [evens, odds]
    perm = np.concatenate(
        [h * HD + np.concatenate([np.arange(0, HD, 2), np.arange(1, HD, 2)])
         for h in range(H)])

    cosP = np.ascontiguousarray(
        np.concatenate([cos_u.T, cos_u.T], axis=0)).astype(
            ml_dtypes.bfloat16)                              # (128, s)
    sinSw = np.ascontiguousarray(
        np.concatenate([sin_u.T, -sin_u.T], axis=0)).astype(
            ml_dtypes.bfloat16)                              # (128, s)

    live, pats = _classify_mask(mask)
    onesb = np.ones((P, 1), ml_dtypes.bfloat16)
    onesf = np.ones((1, P), np.float32)
    mblk = np.ascontiguousarray(pats.transpose(1, 0, 2)).astype(
        ml_dtypes.bfloat16)

    in_maps = []
    for b in range(B):
        xTp = _pre_x(np.ascontiguousarray(x[b].T))
        for g in range(GROUPS):
            rows = perm[g * C:(g + 1) * C]
            in_maps.append({
                "xT": xTp,
                "wqT": _pre_wqk(wq[rows, :].T),
                "wkT": _pre_wqk(wk[rows, :].T),
                "wvT": _pre_w(wv[g * C:(g + 1) * C, :].T),
                "woT": _pre_w(wo[:, g * C:(g + 1) * C].T),
                "cosP": cosP,
                "sinSw": sinSw,
                "mblk": mblk,
                "onesb": onesb,
                "onesf": onesf,
            })
    return in_maps, live, pats


def _run(inputs, trace=False):
    in_maps, live, pats = _prep_host(inputs)
    key = (pats.shape[0], _live_key(live))
    if key not in _PROGRAM_CACHE:
        _PROGRAM_CACHE[key] = _build(live, pats.shape[0])
    nc = _PROGRAM_CACHE[key]
    res = bass_utils.run_bass_kernel_spmd(
        nc, in_maps, core_ids=list(range(NCORES)), trace=trace)
    out = np.zeros((B, S, D), np.float32)
    for b in range(B):
        acc = res.results[b * GROUPS]["out"].astype(np.float32)
        for g in range(1, GROUPS):
            acc += res.results[b * GROUPS + g]["out"].astype(np.float32)
        out[b] = acc
    return out, res


def kernel(**inputs):
    out, _ = _run(inputs, trace=False)
    return out


# revision 24
# speedup vs baseline: 1.2117x; 1.1742x over previous
"""Trainium2 Bass kernel for nn_Attention_51307679318359.

Multi-head attention (B=2, S=2048, D=2048, H=16, HD=128) with RoPE and an
additive mask, sharded over 8 NeuronCores as (batch x head-group): each core
computes 1 batch and 4 heads (512 channels), producing a partial output that
the host sums over head-groups.

All matmul operands are bf16 (fast weight load, half the HBM traffic); PSUM
accumulation stays fp32, so accuracy is well inside the 2e-2 gate.  Q/K/V and
the attention output live entirely in SBUF -- no DRAM spills.

Per-core dataflow (activations transposed, channels on partitions):
  Warmup: ~3.5us of junk matmuls on a memset tile so the PE HAM clock-gate is
  released (2.4 GHz) by the time the first weight/x pieces land; the startup
  DMAs are split into fine pieces across 4 queues so the first real matmul
  issues ~7us in.
  Phase A: QT = wq' @ xT, KT = wk' @ xT with rotate-half permuted weights,
  RoPE applied on the PSUM output (ACT copy + DVE muls); V = xT' @ wv.
  Phase B+C (fused): per (sq-chunk, head), scoresT tiles are matmul'd in
  PAIRS into a 2-bank PSUM tile so one ACT exp covers both (halves the
  per-instruction exp overhead); multiplicative exp(mask) patterns only on
  the diagonal sub-blocks; AV + DVE/GpSimd-accumulated denominator; deferred
  normalization folded into the PSUM->SBUF evacuation (at_ps * bc broadcast).
  The out-projection (C) work of chunk jq-1 is interleaved MM-by-MM into
  chunk jq's score/AV stream, so the PE never idles on exp latency and the
  old phase-C window disappears entirely.
"""

import math

import numpy as np
import ml_dtypes

import concourse.bass as bass
import concourse.mybir as mybir
import concourse.tile as tile
from concourse import bacc
from concourse import bass_utils

F32 = mybir.dt.float32
F32R = mybir.dt.float32r
BF16 = mybir.dt.bfloat16
ADD = mybir.AluOpType.add
MULT = mybir.AluOpType.mult

B, S, D = 2, 2048, 2048
H, HD = 16, 128
NCORES = 8
GROUPS = NCORES // B          # 4 head-groups
HPG = H // GROUPS             # 4 heads per group
C = HPG * HD                  # 512 per-core channels
P = 128
CH = 512                      # phase-A s-chunk width
SQ = 512                      # phase-B sq-chunk width
NKT = D // P                  # 16 k-tiles
NKB = 4                       # k-blocks
NKI = NKT // NKB              # k-tiles per block
NCT = C // P                  # 4 column tiles (= heads per group)
SCALE = 1.0 / math.sqrt(HD)
NEG_THRESH = -1e8             # "masked out" threshold

N_WARM = 40                   # junk warmup matmuls (N=64)
SKEWP = 2                     # item lookahead between front and back stages
EVAC_DELAY = 3                # items after a head's last back before bc+evac
C_QUOTA = 3                   # max C-ops popped per item step
C_RESERVE = 16                 # C-ops held back as tail filler
FUSED_EVAC = True             # DVE TT with two PSUM sources (at_ps * bc_ps)
GP_PATTERNS = False            # pattern muls on GpSimd instead of DVE
GP_NARROW_ADDS = False         # narrow acc adds on GpSimd

_PROGRAM_CACHE = {}


def _pre_wqk(wT):
    """(d, c) -> [P, kb, ct, ki, 128] k-block-major contiguous bf16."""
    a = wT.reshape(NKB, NKI, P, NCT, P)              # [kb, ki, p, ct, col]
    return np.ascontiguousarray(a.transpose(2, 0, 3, 1, 4)).astype(
        ml_dtypes.bfloat16)


def _pre_w(wT):
    """(d, c) -> (128, d//128, c) partition-major contiguous bf16."""
    d, c = wT.shape
    return np.ascontiguousarray(
        wT.reshape(d // P, P, c).transpose(1, 0, 2)).astype(ml_dtypes.bfloat16)


def _pre_x(xT):
    """(d, s) -> (s//CH, 128, d//128, CH) chunk-major contiguous bf16."""
    d, s = xT.shape
    return np.ascontiguousarray(
        xT.reshape(d // P, P, s // CH, CH).transpose(2, 1, 0, 3)).astype(
            ml_dtypes.bfloat16)


def _classify_mask(mask):
    """Classify transposed-mask 128x128 sub-blocks per (sq-chunk jq, sk-tile i,
    sub-col m).  Returns (live, patterns):
      live[jq] = list of (i, c0, c1, [(m, pat_idx), ...]): per live sk tile,
        the live column range [c0, c1) within the chunk and the patterned
        128-col sub-blocks; the first entry always has c0 == 0 and c1 == SQ.
      patterns: (nblk, 128, 128) f32 holding exp(maskT sub-block).
    """
    maskT = np.ascontiguousarray(mask.T)
    s = mask.shape[0]
    n_j = s // SQ
    n_i = s // P
    n_m = SQ // P
    patterns = []
    pat_idx = {}

    def blk_of(jq, i, m):
        return maskT[i * P:(i + 1) * P, jq * SQ + m * P:jq * SQ + (m + 1) * P]

    def add_pattern(blk):
        key = blk.tobytes()
        if key not in pat_idx:
            pat_idx[key] = len(patterns)
            with np.errstate(over='ignore'):
                patterns.append(np.exp(blk.astype(np.float64)).astype(np.float32))
        return pat_idx[key]

    cls = {}
    for jq in range(n_j):
        for i in range(n_i):
            for m in range(n_m):
                blk = blk_of(jq, i, m)
                if np.all(blk == 0.0):
                    cls[(jq, i, m)] = 'plain'
                elif np.all(blk <= NEG_THRESH):
                    cls[(jq, i, m)] = 'skip'
                else:
                    cls[(jq, i, m)] = add_pattern(blk)

    def full_rows(jq):
        rows = []
        for i in range(n_i):
            pats = []
            for m in range(n_m):
                c = cls[(jq, i, m)]
                if c == 'skip':
                    pats.append((m, add_pattern(blk_of(jq, i, m))))
                elif isinstance(c, int):
                    pats.append((m, c))
            rows.append((i, 0, SQ, pats))
        return rows

    live = {}
    for jq in range(n_j):
        rows = []
        for i in range(n_i):
            ms = [m for m in range(n_m) if cls[(jq, i, m)] != 'skip']
            if not ms:
                continue
            m0, m1 = min(ms), max(ms)
            pats = []
            for m in range(m0, m1 + 1):
                c = cls[(jq, i, m)]
                if c == 'skip':
                    # interior hole: apply its (all-zero) pattern explicitly
                    pats.append((m, add_pattern(blk_of(jq, i, m))))
                elif isinstance(c, int):
                    pats.append((m, c))
            rows.append((i, m0 * P, (m1 + 1) * P, pats))
        # PSUM accumulation needs a full-width first tile, and softmax needs
        # every column covered; fall back to no narrowing for odd masks.
        cov = np.zeros(SQ, bool)
        for (_, c0, c1, _) in rows:
            cov[c0:c1] = True
        if not rows or not cov.all():
            rows = full_rows(jq)
        elif rows[0][1] != 0 or rows[0][2] != SQ:
            # move a full-width tile to the front if one exists, else fall back
            fw = [r for r in rows if r[1] == 0 and r[2] == SQ]
            if fw:
                rows.remove(fw[0])
                rows.insert(0, fw[0])
            else:
                rows = full_rows(jq)
        live[jq] = rows
    pats = np.stack(patterns, 0).astype(np.float32) if patterns else \
        np.zeros((1, P, P), np.float32)
    return live, pats


def _live_key(live):
    return tuple(sorted(
        (jq, tuple((i, c0, c1, tuple(p)) for (i, c0, c1, p) in rows))
        for jq, rows in live.items()))


def _make_items(rows):
    """Group a head-chunk's rows into exp-batchable items.

    Row 0 (full width) stays single: its exp writes the denominator
    accumulator tile directly.  Remaining rows are paired so one ACT exp
    covers a 2-bank PSUM tile; a trailing odd row stays single."""
    items = [[rows[0]]]
    k = 1
    while k < len(rows):
        if k + 1 < len(rows):
            items.append([rows[k], rows[k + 1]])
            k += 2
        else:
            items.append([rows[k]])
            k += 1
    return items


def _build(live, nblk, s=S, d=D):
    """Build + compile the per-core SPMD program."""
    n_j = s // SQ
    n_i = s // P
    n_ch = s // CH
    nd = d // SQ

    nc = bacc.Bacc("TRN2", target_bir_lowering=False, debug=False)
    xT = nc.dram_tensor("xT", (n_ch, P, NKT, CH), BF16, kind="ExternalInput")
    wqT = nc.dram_tensor("wqT", (P, NKB, NCT, NKI, P), BF16,
                         kind="ExternalInput")
    wkT = nc.dram_tensor("wkT", (P, NKB, NCT, NKI, P), BF16,
                         kind="ExternalInput")
    wvT = nc.dram_tensor("wvT", (P, NKT, C), BF16, kind="ExternalInput")
    woT = nc.dram_tensor("woT", (P, HPG, d), BF16, kind="ExternalInput")
    cosP = nc.dram_tensor("cosP", (HD, s), BF16, kind="ExternalInput")
    sinSw = nc.dram_tensor("sinSw", (HD, s), BF16, kind="ExternalInput")
    mblk = nc.dram_tensor("mblk", (P, nblk, P), BF16, kind="ExternalInput")
    onesb = nc.dram_tensor("onesb", (P, 1), BF16, kind="ExternalInput")
    onesf = nc.dram_tensor("onesf", (1, P), F32, kind="ExternalInput")
    out = nc.dram_tensor("out", (s, d), BF16, kind="ExternalOutput")

    with tile.TileContext(nc) as tc:
        with tc.tile_pool(name="const", bufs=1) as const:
            ones_b = const.tile([P, 1], BF16)
            ones_f = const.tile([1, P], F32R)
            mblk_t = const.tile([P, nblk, P], BF16)

            qh_sb = const.tile([P, HPG, s], BF16)
            kh_sb = const.tile([P, HPG, s], BF16)
            v_sb = const.tile([P, n_i, C], BF16)
            attn_t = const.tile([P, HPG, s], BF16)
            wo_t = const.tile([P, HPG, d], BF16)

            # ---------------- Phase A: QKV projections + RoPE ----------------
            with tc.tile_pool(name="wres", bufs=1) as wres, \
                 tc.tile_pool(name="xc", bufs=3) as xcp, \
                 tc.tile_pool(name="trig", bufs=1) as trig, \
                 tc.tile_pool(name="ptmp", bufs=2) as ptmp, \
                 tc.tile_pool(name="psA", bufs=6, space="PSUM") as psA, \
                 tc.tile_pool(name="psV", bufs=2, space="PSUM") as psV:
                wq_t = wres.tile([P, NKB, NCT, NKI, P], BF16)
                wk_t = wres.tile([P, NKB, NCT, NKI, P], BF16)
                wv_t = wres.tile([P, NKT, C], BF16)
                junk = wres.tile([P, 64], BF16)
                dum1 = wres.tile([1, P], F32)
                dum2 = wres.tile([P, P], F32)
                cos_t = trig.tile([P, s], BF16)
                sin_t = trig.tile([P, s], BF16)

                # PE warmup: junk matmuls on a memset tile release the HAM
                # clock gate (1.2 -> 2.4 GHz) before the first real matmul
                nc.gpsimd.memset(junk[:], 0)
                nc.gpsimd.memset(dum1[:], 1)
                psW = psA.tile([P, CH], F32, tag="ps_qk", name="ps_warm")
                for _ in range(N_WARM):
                    nc.tensor.matmul(psW[0:64, 0:64], junk[:], junk[:],
                                     start=True, stop=True)

                # startup (3 DMA queues: sync, scalar, gpsimd).  The first
                # matmul group needs only the wq[kb0,ct0] piece and xc0
                # k-tile 0, so sync/scalar lead with 128KB pieces in need
                # order; wk + trig ride gpsimd (K starts ~14us after Q).
                for ct in range(NCT):
                    nc.sync.dma_start(wq_t[:, 0, ct], wqT[:, 0, ct])
                for kb in range(1, NKB):
                    nc.sync.dma_start(wq_t[:, kb], wqT[:, kb])
                xc0 = xcp.tile([P, NKT, CH], BF16, tag="xc")
                for k in range(NKI):
                    nc.scalar.dma_start(xc0[:, k:k + 1, :], xT[0, :, k:k + 1, :])
                for kb in range(1, NKB):
                    nc.scalar.dma_start(xc0[:, kb * NKI:(kb + 1) * NKI, :],
                                        xT[0, :, kb * NKI:(kb + 1) * NKI, :])
                nc.gpsimd.dma_start(cos_t[:], cosP[:])
                nc.gpsimd.dma_start(sin_t[:], sinSw[:])
                nc.gpsimd.dma_start(mblk_t[:], mblk[:])
                nc.gpsimd.dma_start(ones_b[:], onesb[:])
                nc.gpsimd.dma_start(ones_f[:], onesf[:].bitcast(F32R))
                for ct in range(NCT):
                    nc.gpsimd.dma_start(wk_t[:, 0, ct], wkT[:, 0, ct])
                for kb in range(1, NKB):
                    nc.gpsimd.dma_start(wk_t[:, kb], wkT[:, kb])
                nc.gpsimd.dma_start(wo_t[:], woT[:])
                # prime the GpSimd custom-op program (PartitionBroadcast's
                # first use pays a ~7us Q7 library load; do it under phase A)
                nc.gpsimd.partition_broadcast(dum2[:], dum1[:])

                def rope_store(ps, dst, sl):
                    # rotate-half RoPE on a finished PSUM tile -> bf16 SBUF
                    pc = ptmp.tile([P, CH], F32, tag="pc")
                    nc.scalar.activation(
                        pc[:], ps[:], mybir.ActivationFunctionType.Copy)
                    t1 = ptmp.tile([P, CH], F32, tag="t1")
                    nc.vector.tensor_tensor(t1[:], pc[:], cos_t[:, sl], MULT)
                    t2 = ptmp.tile([P, CH], F32, tag="t2")
                    nc.vector.tensor_tensor(
                        t2[0:64, :], pc[64:128, :], sin_t[64:128, sl], MULT)
                    nc.vector.tensor_tensor(
                        t2[64:128, :], pc[0:64, :], sin_t[0:64, sl], MULT)
                    nc.vector.tensor_tensor(dst, t1[:], t2[:], ADD)

                def v_proj(jv, xcv):
                    for st2 in range(CH // P):
                        st = (jv * CH) // P + st2
                        psv = psV.tile([P, C], F32, tag="ps_v")
                        for k in range(NKT):
                            nc.tensor.matmul(
                                psv[:], xcv[:, k, st2 * P:(st2 + 1) * P],
                                wv_t[:, k, :],
                                start=(k == 0), stop=(k == NKT - 1))
                        # ACT copy keeps the DVE queue clear for phase B
                        nc.scalar.activation(
                            v_sb[:, st, :], psv[:],
                            mybir.ActivationFunctionType.Copy)

                prev = None
                nxt = None
                for j in range(n_ch):
                    sl = slice(j * CH, (j + 1) * CH)
                    xc = xc0 if j == 0 else nxt
                    # prefetch the next chunk a full iteration ahead, split
                    # across the scalar and sync queues so neither starves
                    if j + 1 < n_ch:
                        nxt = xcp.tile([P, NKT, CH], BF16, tag="xc")
                        for kb in range(NKB):
                            eng = nc.scalar if kb % 2 == 0 else nc.sync
                            eng.dma_start(
                                nxt[:, kb * NKI:(kb + 1) * NKI, :],
                                xT[j + 1, :, kb * NKI:(kb + 1) * NKI, :])
                    if j == 0:
                        # wv rides behind chunk 1's prefetch on both queues
                        nc.scalar.dma_start(wv_t[:, 0:NKT // 2, :],
                                            wvT[:, 0:NKT // 2, :])
                        nc.sync.dma_start(wv_t[:, NKT // 2:, :],
                                          wvT[:, NKT // 2:, :])
                    for (wt, dst) in ((wq_t, qh_sb), (wk_t, kh_sb)):
                        if j == 0:
                            # kb-major: consumes startup DMA pieces in
                            # arrival order
                            pss = [psA.tile([P, CH], F32, tag="ps_qk",
                                            name=f"ps_qk{ct}")
                                   for ct in range(NCT)]
                            for kb in range(NKB):
                                for ct in range(NCT):
                                    for ki in range(NKI):
                                        nc.tensor.matmul(
                                            pss[ct][:], wt[:, kb, ct, ki, :],
                                            xc[:, kb * NKI + ki, :],
                                            start=(kb == 0 and ki == 0),
                                            stop=(kb == NKB - 1 and
                                                  ki == NKI - 1),
                                            skip_group_check=True)
                            for ct in range(NCT):
                                rope_store(pss[ct], dst[:, ct, sl], sl)
                        else:
                            # ct-major: each rope starts right after its own
                            # 16-MM chain, so the DVE queue drains during the
                            # chunk instead of piling up at the A->B boundary
                            for ct in range(NCT):
                                psc = psA.tile([P, CH], F32, tag="ps_qk",
                                               name=f"ps_qk{ct}")
                                for kb in range(NKB):
                                    for ki in range(NKI):
                                        nc.tensor.matmul(
                                            psc[:], wt[:, kb, ct, ki, :],
                                            xc[:, kb * NKI + ki, :],
                                            start=(kb == 0 and ki == 0),
                                            stop=(kb == NKB - 1 and
                                                  ki == NKI - 1),
                                            skip_group_check=True)
                                rope_store(psc, dst[:, ct, sl], sl)
                    if prev is not None:
                        v_proj(j - 1, prev)
                    prev = xc
                v_proj(n_ch - 1, prev)

            # ---------------- Phase B+C fused ----------------
            # Global software pipeline over (chunk jq, head h, item): scores
            # matmuls + pair-batched exp run SKEWP items ahead of the AV /
            # denominator stage, and the out-projection of chunk jq-1 is
            # drip-fed into the stream so the PE never waits on ACT.
            entries = []
            head_last = {}
            for jq in range(n_j):
                rows = live[jq]
                items = _make_items(rows)
                for h in range(HPG):
                    n0 = 0
                    for k2, it in enumerate(items):
                        entries.append(dict(
                            jq=jq, h=h, item=it, n0=n0,
                            first=(k2 == 0), last=(k2 == len(items) - 1),
                            r=len(rows)))
                        n0 += len(it)
                    head_last[(jq, h)] = len(entries) - 1

            evac_at = {}
            for (jq, h), last in head_last.items():
                evac_at.setdefault(last + SKEWP + EVAC_DELAY, []).append((jq, h))
            c_release = {head_last[(jq, HPG - 1)] + SKEWP + EVAC_DELAY + 2: jq
                         for jq in range(n_j - 1)}

            with tc.tile_pool(name="rcp", bufs=3) as rcp, \
                 tc.tile_pool(name="sm", bufs=6) as smp, \
                 tc.tile_pool(name="pr", bufs=SKEWP + 3) as prp, \
                 tc.tile_pool(name="acc", bufs=8) as accp, \
                 tc.tile_pool(name="og", bufs=2) as ogp, \
                 tc.tile_pool(name="psS", bufs=2, space="PSUM") as psS, \
                 tc.tile_pool(name="psAt", bufs=2, space="PSUM") as psAt, \
                 tc.tile_pool(name="psC", bufs=2, space="PSUM") as psC:

                state = {}    # (jq,h) -> dict(at_ps, acc)
                rcs = {}
                c_queue = []
                c_po = {}     # current po tile for the active C dch chain
                c_og = {}     # st -> og tile

                def pat_mul(dst_ap, pidx):
                    if GP_PATTERNS:
                        nc.gpsimd.scalar_tensor_tensor(
                            dst_ap, dst_ap, 1.0, mblk_t[:, pidx, :],
                            MULT, MULT)
                    else:
                        nc.vector.tensor_tensor(
                            dst_ap, dst_ap, mblk_t[:, pidx, :], MULT)

                def emit_front(e):
                    jq, h, it = e['jq'], e['h'], e['item']
                    qb = jq * SQ
                    if e['first']:
                        state[(jq, h)] = dict(
                            acc=accp.tile([P, SQ], BF16, tag="acc", name="acc"))
                    st_ = state[(jq, h)]
                    if e['first']:
                        # row 0: single full-width exp writes the denominator
                        # accumulator directly
                        (i, c0, c1, pats) = it[0]
                        sc = psS.tile([P, 2 * SQ], F32, tag="sc", name="sc")
                        nc.tensor.matmul(
                            sc[:, c0:c1],
                            kh_sb[:, h, i * P:(i + 1) * P],
                            qh_sb[:, h, qb + c0:qb + c1],
                            start=True, stop=True)
                        acc = st_['acc']
                        nc.scalar.activation(
                            acc[:, c0:c1], sc[:, c0:c1],
                            mybir.ActivationFunctionType.Exp, scale=SCALE)
                        for (m, pidx) in pats:
                            pat_mul(acc[:, m * P:(m + 1) * P], pidx)
                        e['src'] = acc
                        e['offs'] = [0]
                        return
                    sc = psS.tile([P, 2 * SQ], F32, tag="sc", name="sc")
                    pr = prp.tile([P, 2 * SQ], BF16, tag="pr", name="pr")
                    for k2, (i, c0, c1, pats) in enumerate(it):
                        off = k2 * SQ
                        nc.tensor.matmul(
                            sc[:, off + c0:off + c1],
                            kh_sb[:, h, i * P:(i + 1) * P],
                            qh_sb[:, h, qb + c0:qb + c1],
                            start=True, stop=True)
                    # one exp per pair when contiguous; split when row B has
                    # a dead prefix so stale PSUM is never exp'd (no
                    # manufactured inf/nan in pr)
                    if len(it) == 2 and it[1][1] > 0:
                        for k2, (i, c0, c1, pats) in enumerate(it):
                            off = k2 * SQ
                            nc.scalar.activation(
                                pr[:, off + c0:off + c1],
                                sc[:, off + c0:off + c1],
                                mybir.ActivationFunctionType.Exp, scale=SCALE)
                    else:
                        c0a = it[0][1]
                        end = (len(it) - 1) * SQ + it[-1][2]
                        nc.scalar.activation(
                            pr[:, c0a:end], sc[:, c0a:end],
                            mybir.ActivationFunctionType.Exp, scale=SCALE)
                    for k2, (i, c0, c1, pats) in enumerate(it):
                        off = k2 * SQ
                        for (m, pidx) in pats:
                            pat_mul(pr[:, off + m * P:off + (m + 1) * P], pidx)
                    e['src'] = pr
                    e['offs'] = [k2 * SQ for k2 in range(len(it))]

                def emit_back(e):
                    jq, h, it = e['jq'], e['h'], e['item']
                    st_ = state[(jq, h)]
                    src = e['src']
                    acc = st_['acc']
                    for k2, (i, c0, c1, pats) in enumerate(it):
                        n = e['n0'] + k2
                        off = e['offs'][k2]
                        if n == 0:
                            st_['at_ps'] = psAt.tile([P, SQ], F32, tag="at", name="at")
                        at_ps = st_['at_ps']
                        nc.tensor.matmul(
                            at_ps[:, c0:c1],
                            v_sb[:, i, h * HD:(h + 1) * HD],
                            src[:, off + c0:off + c1],
                            start=(n == 0), stop=(n == e['r'] - 1),
                            skip_group_check=True)
                        if n > 0:
                            wide = (c1 - c0) > 256
                            if wide or not GP_NARROW_ADDS:
                                nc.vector.tensor_tensor(
                                    acc[:, c0:c1], acc[:, c0:c1],
                                    src[:, off + c0:off + c1], ADD)
                            else:
                                nc.gpsimd.scalar_tensor_tensor(
                                    acc[:, c0:c1], acc[:, c0:c1], 1.0,
                                    src[:, off + c0:off + c1], MULT, ADD)

                def emit_dn(jq, h):
                    acc = state[(jq, h)]['acc']
                    dn_ps = psS.tile([P, 2 * SQ], F32, tag="sc", name="dn_ps")
                    nc.tensor.matmul(dn_ps[0:1, 0:SQ], ones_b[:], acc[:],
                                     start=True, stop=True)
                    dn_sb = smp.tile([1, SQ], F32, tag="dnsb", name="dn_sb")
                    last = (jq == n_j - 1 and h == HPG - 1)
                    if last:
                        nc.vector.tensor_copy(dn_sb[:], dn_ps[0:1, 0:SQ])
                    else:
                        nc.scalar.activation(dn_sb[:], dn_ps[0:1, 0:SQ],
                                             mybir.ActivationFunctionType.Copy)
                    if last:
                        # tail chain gates the final out-projection: skip
                        # the fold hops, reciprocal directly on (1,512)
                        rc1 = smp.tile([1, SQ], F32, tag="rc1", name="rc1d")
                        nc.vector.reciprocal_approx_fast(rc1[:], dn_sb[:])
                    else:
                        # reciprocal cost scales with free-size per lane:
                        # fold the (1,512) row to (4,128) via DMA first
                        dn4 = smp.tile([SQ // P, P], F32, tag="dn4",
                                       name="dn4")
                        nc.gpsimd.dma_start(dn4[:], dn_sb[:])
                        rc4 = smp.tile([SQ // P, P], F32, tag="rc4",
                                       name="rc4")
                        nc.vector.reciprocal_approx_fast(rc4[:], dn4[:])
                        rc1 = smp.tile([1, SQ], F32, tag="rc1", name="rc1")
                        nc.gpsimd.dma_start(rc1[:], rc4[:])
                    # broadcast 1/dn to all partitions on GpSimd so the DVE
                    # evacuation TT has a legal SBUF second operand and the
                    # PE skips the ones-broadcast matmul
                    bc_sb = rcp.tile([P, SQ], F32, tag="rc", name="bc_sb")
                    nc.gpsimd.partition_broadcast(bc_sb[:], rc1[:])
                    rcs[(jq, h)] = bc_sb
                    if jq == 0:
                        # chunk-0 heads are only ~2us apart: evacuate the AV
                        # accumulator unnormalized NOW so psAt never starves;
                        # the normalization TT happens at the usual evac slot
                        at_ps = state[(jq, h)].pop('at_ps')
                        at_sb = accp.tile([P, SQ], BF16, tag="acc",
                                          name="at_sb")
                        nc.vector.tensor_copy(at_sb[:], at_ps[:])
                        state[(jq, h)]['at_sb'] = at_sb

                def emit_evac(jq, h):
                    bc_sb = rcs.pop((jq, h))
                    qsl = slice(jq * SQ, (jq + 1) * SQ)
                    if jq == 0:
                        at_sb = state[(jq, h)].pop('at_sb')
                        nc.vector.tensor_tensor(
                            attn_t[:, h, qsl], at_sb[:], bc_sb[:], MULT)
                    else:
                        at_ps = state[(jq, h)].pop('at_ps')
                        nc.vector.tensor_tensor(
                            attn_t[:, h, qsl], at_ps[:], bc_sb[:], MULT)

                def push_c_ops(jq, final=False):
                    for st2 in range(SQ // P):
                        st = jq * (SQ // P) + st2
                        for dch in range(nd):
                            for ct in range(HPG):
                                c_queue.append((st, dch, ct, final))

                def pop_c(quota):
                    npop = min(quota, len(c_queue))
                    for _ in range(npop):
                        (st, dch, ct, final) = c_queue.pop(0)
                        if ct == 0 and dch == 0:
                            c_og[st] = ogp.tile([P, d], BF16, tag="og", name="og")
                        if ct == 0:
                            c_po[st] = psC.tile([P, SQ], F32, tag="po", name="po")
                        po = c_po[st]
                        nc.tensor.matmul(
                            po[:], attn_t[:, ct, st * P:(st + 1) * P],
                            wo_t[:, ct, dch * SQ:(dch + 1) * SQ],
                            start=(ct == 0), stop=(ct == HPG - 1))
                        if ct != HPG - 1:
                            continue
                        og = c_og[st]
                        osl = slice(dch * SQ, (dch + 1) * SQ)
                        if dch % 4 == 3:
                            nc.scalar.activation(
                                og[:, osl], po[:],
                                mybir.ActivationFunctionType.Copy)
                        else:
                            nc.vector.tensor_copy(og[:, osl], po[:])
                        rsl = slice(st * P, (st + 1) * P)
                        if final:
                            # final chunk: write each dch immediately on a
                            # rotating queue so the tail DMA drains early
                            eng = (nc.sync, nc.gpsimd, nc.scalar,
                                   nc.sync)[(st + dch) % 4]
                            eng.dma_start(out[rsl, osl], og[:, osl])
                        elif dch % 2 == 1:
                            eng = nc.sync if (st + dch) % 4 == 1 else nc.gpsimd
                            eng.dma_start(
                                out[rsl, (dch - 1) * SQ:(dch + 1) * SQ],
                                og[:, (dch - 1) * SQ:(dch + 1) * SQ])

                total = len(entries) + SKEWP
                for idx in range(total):
                    if idx < len(entries):
                        emit_front(entries[idx])
                    bidx = idx - SKEWP
                    if bidx >= 0:
                        e = entries[bidx]
                        emit_back(e)
                        if e['last']:
                            emit_dn(e['jq'], e['h'])
                    for (jq, h) in evac_at.get(idx, []):
                        emit_evac(jq, h)
                    if idx in c_release:
                        push_c_ops(c_release[idx])
                    if len(c_queue) > C_RESERVE:
                        pop_c(min(C_QUOTA, len(c_queue) - C_RESERVE))

                # tail: held-back C ops cover the last rc chain, then the
                # final chunk's evacs and out-projection drain
                pop_c(C_RESERVE)
                for k in sorted(k for k in evac_at if k >= total):
                    for (jq, h) in evac_at[k]:
                        emit_evac(jq, h)
                push_c_ops(n_j - 1, final=True)
                pop_c(len(c_queue))

    nc.compile()
    return nc


def _prep_host(inputs):
    """Shard + transpose the full inputs into 8 per-core input maps."""
    x = np.asarray(inputs["x"], np.float32)
    wq = np.asarray(inputs["wq"], np.float32)
    wk = np.asarray(inputs["wk"], np.float32)
    wv = np.asarray(inputs["wv"], np.float32)
    wo = np.asarray(inputs["wo"], np.float32)
    cos = np.asarray(inputs["cos"], np.float32)
    sin = np.asarray(inputs["sin"], np.float32)
    mask = np.asarray(inputs["mask"], np.float32)
    start_p = int(inputs["start_p"])

    s = x.shape[1]
    cos_u = cos[start_p:start_p + s]          # (s, HD/2)
    sin_u = sin[start_p:start_p + s]

    # rotate-half channel permutation within each head: [evens, odds]
    perm = np.concatenate(
        [h * HD + np.concatenate([np.arange(0, HD, 2), np.arange(1, HD, 2)])
         for h in range(H)])

    cosP = np.ascontiguousarray(
        np.concatenate([cos_u.T, cos_u.T], axis=0)).astype(
            ml_dtypes.bfloat16)                              # (128, s)
    sinSw = np.ascontiguousarray(
        np.concatenate([sin_u.T, -sin_u.T], axis=0)).astype(
            ml_dtypes.bfloat16)                              # (128, s)

    live, pats = _classify_mask(mask)
    onesb = np.ones((P, 1), ml_dtypes.bfloat16)
    onesf = np.ones((1, P), np.float32)
    mblk = np.ascontiguousarray(pats.transpose(1, 0, 2)).astype(
        ml_dtypes.bfloat16)

    in_maps = []
    for b in range(B):
        xTp = _pre_x(np.ascontiguousarray(x[b].T))
        for g in range(GROUPS):
            rows = perm[g * C:(g + 1) * C]
            in_maps.append({
                "xT": xTp,
                "wqT": _pre_wqk(wq[rows, :].T),
                "wkT": _pre_wqk(wk[rows, :].T),
                "wvT": _pre_w(wv[g * C:(g + 1) * C, :].T),
                "woT": _pre_w(wo[:, g * C:(g + 1) * C].T),
                "cosP": cosP,
                "sinSw": sinSw,
                "mblk": mblk,
                "onesb": onesb,
                "onesf": onesf,
            })
    return in_maps, live, pats


def _run(inputs, trace=False):
    in_maps, live, pats = _prep_host(inputs)
    key = (pats.shape[0], _live_key(live))
    if key not in _PROGRAM_CACHE:
        _PROGRAM_CACHE[key] = _build(live, pats.shape[0])
    nc = _PROGRAM_CACHE[key]
    res = bass_utils.run_bass_kernel_spmd(
        nc, in_maps, core_ids=list(range(NCORES)), trace=trace)
    out = np.zeros((B, S, D), np.float32)
    for b in range(B):
        acc = res.results[b * GROUPS]["out"].astype(np.float32)
        for g in range(1, GROUPS):
            acc += res.results[b * GROUPS + g]["out"].astype(np.float32)
        out[b] = acc
    return out, res


def kernel(**inputs):
    out, _ = _run(inputs, trace=False)
    return out


# revision 25
# speedup vs baseline: 1.2147x; 1.0024x over previous
"""Trainium2 Bass kernel for nn_Attention_51307679318359.

Multi-head attention (B=2, S=2048, D=2048, H=16, HD=128) with RoPE and an
additive mask, sharded over 8 NeuronCores as (batch x head-group): each core
computes 1 batch and 4 heads (512 channels), producing a partial output that
the host sums over head-groups.

All matmul operands are bf16 (fast weight load, half the HBM traffic); PSUM
accumulation stays fp32, so accuracy is well inside the 2e-2 gate.  Q/K/V and
the attention output live entirely in SBUF -- no DRAM spills.

Per-core dataflow (activations transposed, channels on partitions):
  Warmup: ~4us of junk matmuls on a memset tile release the PE HAM clock gate
  (1.2 -> 2.4 GHz) before the first weight/x pieces land; startup DMAs lead
  with 128KB pieces in need-order on the sync/scalar queues (wq | x), with
  trig + wk + wo on gpsimd and wv split across sync/scalar behind chunk 1.
  A GpSimd PartitionBroadcast is primed under phase A (its first use pays a
  ~7us Q7 program load).
  Phase A: QT = wq' @ xT, KT = wk' @ xT with rotate-half permuted weights,
  RoPE applied on the PSUM output (ACT copy + DVE muls, ct-major after chunk
  0 so the DVE queue drains before phase B); V = xT' @ wv with ACT-copy
  evacuation.
  Phase B+C (fused): one global software pipeline over (sq-chunk, head,
  item).  Score tiles are matmul'd in pairs into a 2-bank PSUM tile so one
  ACT exp covers both (halving exp instruction overhead); exp(mask) patterns
  multiply only the diagonal sub-blocks; AV accumulates in PSUM while the
  denominator accumulates on the DVE in bf16; 1/denominator via
  reciprocal_approx_fast on a DMA-folded (4,128) row, partition-broadcast to
  all lanes on GpSimd, and folded into the PSUM->SBUF evacuation TT.  The
  out-projection of chunk jq-1 is drip-fed MM-by-MM into chunk jq's
  score/AV stream (with a reserved tail batch covering the last rc chain),
  so the PE never idles on exp latency and the old phase-C window
  disappears.  Output rows stream to HBM per dch-pair on rotating queues,
  per-dch for the final chunk.
"""

import math

import numpy as np
import ml_dtypes

import concourse.bass as bass
import concourse.mybir as mybir
import concourse.tile as tile
from concourse import bacc
from concourse import bass_utils

F32 = mybir.dt.float32
F32R = mybir.dt.float32r
BF16 = mybir.dt.bfloat16
ADD = mybir.AluOpType.add
MULT = mybir.AluOpType.mult

B, S, D = 2, 2048, 2048
H, HD = 16, 128
NCORES = 8
GROUPS = NCORES // B          # 4 head-groups
HPG = H // GROUPS             # 4 heads per group
C = HPG * HD                  # 512 per-core channels
P = 128
CH = 512                      # phase-A s-chunk width
SQ = 512                      # phase-B sq-chunk width
NKT = D // P                  # 16 k-tiles
NKB = 4                       # k-blocks
NKI = NKT // NKB              # k-tiles per block
NCT = C // P                  # 4 column tiles (= heads per group)
SCALE = 1.0 / math.sqrt(HD)
NEG_THRESH = -1e8             # "masked out" threshold

N_WARM = 40                   # junk warmup matmuls (N=64)
SKEWP = 2                     # item lookahead between front and back stages
EVAC_DELAY = 3                # items after a head's last back before bc+evac
C_QUOTA = 3                   # max C-ops popped per item step
C_RESERVE = 16                 # C-ops held back as tail filler
GP_PATTERNS = False            # pattern muls on GpSimd instead of DVE
GP_NARROW_ADDS = False         # narrow acc adds on GpSimd

_PROGRAM_CACHE = {}


def _pre_wqk(wT):
    """(d, c) -> [P, kb, ct, ki, 128] k-block-major contiguous bf16."""
    a = wT.reshape(NKB, NKI, P, NCT, P)              # [kb, ki, p, ct, col]
    return np.ascontiguousarray(a.transpose(2, 0, 3, 1, 4)).astype(
        ml_dtypes.bfloat16)


def _pre_w(wT):
    """(d, c) -> (128, d//128, c) partition-major contiguous bf16."""
    d, c = wT.shape
    return np.ascontiguousarray(
        wT.reshape(d // P, P, c).transpose(1, 0, 2)).astype(ml_dtypes.bfloat16)


def _pre_x(xT):
    """(d, s) -> (s//CH, 128, d//128, CH) chunk-major contiguous bf16."""
    d, s = xT.shape
    return np.ascontiguousarray(
        xT.reshape(d // P, P, s // CH, CH).transpose(2, 1, 0, 3)).astype(
            ml_dtypes.bfloat16)


def _classify_mask(mask):
    """Classify transposed-mask 128x128 sub-blocks per (sq-chunk jq, sk-tile i,
    sub-col m).  Returns (live, patterns):
      live[jq] = list of (i, c0, c1, [(m, pat_idx), ...]): per live sk tile,
        the live column range [c0, c1) within the chunk and the patterned
        128-col sub-blocks; the first entry always has c0 == 0 and c1 == SQ.
      patterns: (nblk, 128, 128) f32 holding exp(maskT sub-block).
    """
    maskT = np.ascontiguousarray(mask.T)
    s = mask.shape[0]
    n_j = s // SQ
    n_i = s // P
    n_m = SQ // P
    patterns = []
    pat_idx = {}

    def blk_of(jq, i, m):
        return maskT[i * P:(i + 1) * P, jq * SQ + m * P:jq * SQ + (m + 1) * P]

    def add_pattern(blk):
        key = blk.tobytes()
        if key not in pat_idx:
            pat_idx[key] = len(patterns)
            with np.errstate(over='ignore'):
                patterns.append(np.exp(blk.astype(np.float64)).astype(np.float32))
        return pat_idx[key]

    cls = {}
    for jq in range(n_j):
        for i in range(n_i):
            for m in range(n_m):
                blk = blk_of(jq, i, m)
                if np.all(blk == 0.0):
                    cls[(jq, i, m)] = 'plain'
                elif np.all(blk <= NEG_THRESH):
                    cls[(jq, i, m)] = 'skip'
                else:
                    cls[(jq, i, m)] = add_pattern(blk)

    def full_rows(jq):
        rows = []
        for i in range(n_i):
            pats = []
            for m in range(n_m):
                c = cls[(jq, i, m)]
                if c == 'skip':
                    pats.append((m, add_pattern(blk_of(jq, i, m))))
                elif isinstance(c, int):
                    pats.append((m, c))
            rows.append((i, 0, SQ, pats))
        return rows

    live = {}
    for jq in range(n_j):
        rows = []
        for i in range(n_i):
            ms = [m for m in range(n_m) if cls[(jq, i, m)] != 'skip']
            if not ms:
                continue
            m0, m1 = min(ms), max(ms)
            pats = []
            for m in range(m0, m1 + 1):
                c = cls[(jq, i, m)]
                if c == 'skip':
                    # interior hole: apply its (all-zero) pattern explicitly
                    pats.append((m, add_pattern(blk_of(jq, i, m))))
                elif isinstance(c, int):
                    pats.append((m, c))
            rows.append((i, m0 * P, (m1 + 1) * P, pats))
        # PSUM accumulation needs a full-width first tile, and softmax needs
        # every column covered; fall back to no narrowing for odd masks.
        cov = np.zeros(SQ, bool)
        for (_, c0, c1, _) in rows:
            cov[c0:c1] = True
        if not rows or not cov.all():
            rows = full_rows(jq)
        elif rows[0][1] != 0 or rows[0][2] != SQ:
            # move a full-width tile to the front if one exists, else fall back
            fw = [r for r in rows if r[1] == 0 and r[2] == SQ]
            if fw:
                rows.remove(fw[0])
                rows.insert(0, fw[0])
            else:
                rows = full_rows(jq)
        live[jq] = rows
    pats = np.stack(patterns, 0).astype(np.float32) if patterns else \
        np.zeros((1, P, P), np.float32)
    return live, pats


def _live_key(live):
    return tuple(sorted(
        (jq, tuple((i, c0, c1, tuple(p)) for (i, c0, c1, p) in rows))
        for jq, rows in live.items()))


def _make_items(rows):
    """Group a head-chunk's rows into exp-batchable items.

    Row 0 (full width) stays single: its exp writes the denominator
    accumulator tile directly.  Remaining rows are paired so one ACT exp
    covers a 2-bank PSUM tile; a trailing odd row stays single."""
    items = [[rows[0]]]
    k = 1
    while k < len(rows):
        if k + 1 < len(rows):
            items.append([rows[k], rows[k + 1]])
            k += 2
        else:
            items.append([rows[k]])
            k += 1
    return items


def _build(live, nblk, s=S, d=D):
    """Build + compile the per-core SPMD program."""
    n_j = s // SQ
    n_i = s // P
    n_ch = s // CH
    nd = d // SQ

    nc = bacc.Bacc("TRN2", target_bir_lowering=False, debug=False)
    xT = nc.dram_tensor("xT", (n_ch, P, NKT, CH), BF16, kind="ExternalInput")
    wqT = nc.dram_tensor("wqT", (P, NKB, NCT, NKI, P), BF16,
                         kind="ExternalInput")
    wkT = nc.dram_tensor("wkT", (P, NKB, NCT, NKI, P), BF16,
                         kind="ExternalInput")
    wvT = nc.dram_tensor("wvT", (P, NKT, C), BF16, kind="ExternalInput")
    woT = nc.dram_tensor("woT", (P, HPG, d), BF16, kind="ExternalInput")
    cosP = nc.dram_tensor("cosP", (HD, s), BF16, kind="ExternalInput")
    sinSw = nc.dram_tensor("sinSw", (HD, s), BF16, kind="ExternalInput")
    mblk = nc.dram_tensor("mblk", (P, nblk, P), BF16, kind="ExternalInput")
    onesb = nc.dram_tensor("onesb", (P, 1), BF16, kind="ExternalInput")
    onesf = nc.dram_tensor("onesf", (1, P), F32, kind="ExternalInput")
    out = nc.dram_tensor("out", (s, d), BF16, kind="ExternalOutput")

    with tile.TileContext(nc) as tc:
        with tc.tile_pool(name="const", bufs=1) as const:
            ones_b = const.tile([P, 1], BF16)
            ones_f = const.tile([1, P], F32R)
            mblk_t = const.tile([P, nblk, P], BF16)

            qh_sb = const.tile([P, HPG, s], BF16)
            kh_sb = const.tile([P, HPG, s], BF16)
            v_sb = const.tile([P, n_i, C], BF16)
            attn_t = const.tile([P, HPG, s], BF16)
            wo_t = const.tile([P, HPG, d], BF16)

            # ---------------- Phase A: QKV projections + RoPE ----------------
            with tc.tile_pool(name="wres", bufs=1) as wres, \
                 tc.tile_pool(name="xc", bufs=3) as xcp, \
                 tc.tile_pool(name="trig", bufs=1) as trig, \
                 tc.tile_pool(name="ptmp", bufs=2) as ptmp, \
                 tc.tile_pool(name="psA", bufs=6, space="PSUM") as psA, \
                 tc.tile_pool(name="psV", bufs=2, space="PSUM") as psV:
                wq_t = wres.tile([P, NKB, NCT, NKI, P], BF16)
                wk_t = wres.tile([P, NKB, NCT, NKI, P], BF16)
                wv_t = wres.tile([P, NKT, C], BF16)
                junk = wres.tile([P, 64], BF16)
                dum1 = wres.tile([1, P], F32)
                dum2 = wres.tile([P, P], F32)
                cos_t = trig.tile([P, s], BF16)
                sin_t = trig.tile([P, s], BF16)

                # PE warmup: junk matmuls on a memset tile release the HAM
                # clock gate (1.2 -> 2.4 GHz) before the first real matmul
                nc.gpsimd.memset(junk[:], 0)
                nc.gpsimd.memset(dum1[:], 1)
                psW = psA.tile([P, CH], F32, tag="ps_qk", name="ps_warm")
                for _ in range(N_WARM):
                    nc.tensor.matmul(psW[0:64, 0:64], junk[:], junk[:],
                                     start=True, stop=True)

                # startup (3 DMA queues: sync, scalar, gpsimd).  The first
                # matmul group needs only the wq[kb0,ct0] piece and xc0
                # k-tile 0, so sync/scalar lead with 128KB pieces in need
                # order; wk + trig ride gpsimd (K starts ~14us after Q).
                for ct in range(NCT):
                    nc.sync.dma_start(wq_t[:, 0, ct], wqT[:, 0, ct])
                for kb in range(1, NKB):
                    nc.sync.dma_start(wq_t[:, kb], wqT[:, kb])
                xc0 = xcp.tile([P, NKT, CH], BF16, tag="xc")
                for k in range(NKI):
                    nc.scalar.dma_start(xc0[:, k:k + 1, :], xT[0, :, k:k + 1, :])
                for kb in range(1, NKB):
                    nc.scalar.dma_start(xc0[:, kb * NKI:(kb + 1) * NKI, :],
                                        xT[0, :, kb * NKI:(kb + 1) * NKI, :])
                nc.gpsimd.dma_start(cos_t[:], cosP[:])
                nc.gpsimd.dma_start(sin_t[:], sinSw[:])
                nc.gpsimd.dma_start(mblk_t[:], mblk[:])
                nc.gpsimd.dma_start(ones_b[:], onesb[:])
                nc.gpsimd.dma_start(ones_f[:], onesf[:].bitcast(F32R))
                for ct in range(NCT):
                    nc.gpsimd.dma_start(wk_t[:, 0, ct], wkT[:, 0, ct])
                for kb in range(1, NKB):
                    nc.gpsimd.dma_start(wk_t[:, kb], wkT[:, kb])
                nc.gpsimd.dma_start(wo_t[:], woT[:])
                # prime the GpSimd custom-op program (PartitionBroadcast's
                # first use pays a ~7us Q7 library load; do it under phase A)
                nc.gpsimd.partition_broadcast(dum2[:], dum1[:])

                def rope_store(ps, dst, sl):
                    # rotate-half RoPE on a finished PSUM tile -> bf16 SBUF
                    pc = ptmp.tile([P, CH], F32, tag="pc")
                    nc.scalar.activation(
                        pc[:], ps[:], mybir.ActivationFunctionType.Copy)
                    t1 = ptmp.tile([P, CH], F32, tag="t1")
                    nc.vector.tensor_tensor(t1[:], pc[:], cos_t[:, sl], MULT)
                    t2 = ptmp.tile([P, CH], F32, tag="t2")
                    nc.vector.tensor_tensor(
                        t2[0:64, :], pc[64:128, :], sin_t[64:128, sl], MULT)
                    nc.vector.tensor_tensor(
                        t2[64:128, :], pc[0:64, :], sin_t[0:64, sl], MULT)
                    nc.vector.tensor_tensor(dst, t1[:], t2[:], ADD)

                def v_proj(jv, xcv):
                    for st2 in range(CH // P):
                        st = (jv * CH) // P + st2
                        psv = psV.tile([P, C], F32, tag="ps_v")
                        for k in range(NKT):
                            nc.tensor.matmul(
                                psv[:], xcv[:, k, st2 * P:(st2 + 1) * P],
                                wv_t[:, k, :],
                                start=(k == 0), stop=(k == NKT - 1))
                        # ACT copy keeps the DVE queue clear for phase B
                        nc.scalar.activation(
                            v_sb[:, st, :], psv[:],
                            mybir.ActivationFunctionType.Copy)

                prev = None
                nxt = None
                for j in range(n_ch):
                    sl = slice(j * CH, (j + 1) * CH)
                    xc = xc0 if j == 0 else nxt
                    # prefetch the next chunk a full iteration ahead, split
                    # across the scalar and sync queues so neither starves
                    if j + 1 < n_ch:
                        nxt = xcp.tile([P, NKT, CH], BF16, tag="xc")
                        for kb in range(NKB):
                            eng = nc.scalar if kb % 2 == 0 else nc.sync
                            eng.dma_start(
                                nxt[:, kb * NKI:(kb + 1) * NKI, :],
                                xT[j + 1, :, kb * NKI:(kb + 1) * NKI, :])
                    if j == 0:
                        # wv rides behind chunk 1's prefetch on both queues
                        nc.scalar.dma_start(wv_t[:, 0:NKT // 2, :],
                                            wvT[:, 0:NKT // 2, :])
                        nc.sync.dma_start(wv_t[:, NKT // 2:, :],
                                          wvT[:, NKT // 2:, :])
                    for (wt, dst) in ((wq_t, qh_sb), (wk_t, kh_sb)):
                        if j == 0:
                            # kb-major: consumes startup DMA pieces in
                            # arrival order
                            pss = [psA.tile([P, CH], F32, tag="ps_qk",
                                            name=f"ps_qk{ct}")
                                   for ct in range(NCT)]
                            for kb in range(NKB):
                                for ct in range(NCT):
                                    for ki in range(NKI):
                                        nc.tensor.matmul(
                                            pss[ct][:], wt[:, kb, ct, ki, :],
                                            xc[:, kb * NKI + ki, :],
                                            start=(kb == 0 and ki == 0),
                                            stop=(kb == NKB - 1 and
                                                  ki == NKI - 1),
                                            skip_group_check=True)
                            for ct in range(NCT):
                                rope_store(pss[ct], dst[:, ct, sl], sl)
                        else:
                            # ct-major: each rope starts right after its own
                            # 16-MM chain, so the DVE queue drains during the
                            # chunk instead of piling up at the A->B boundary
                            for ct in range(NCT):
                                psc = psA.tile([P, CH], F32, tag="ps_qk",
                                               name=f"ps_qk{ct}")
                                for kb in range(NKB):
                                    for ki in range(NKI):
                                        nc.tensor.matmul(
                                            psc[:], wt[:, kb, ct, ki, :],
                                            xc[:, kb * NKI + ki, :],
                                            start=(kb == 0 and ki == 0),
                                            stop=(kb == NKB - 1 and
                                                  ki == NKI - 1),
                                            skip_group_check=True)
                                rope_store(psc, dst[:, ct, sl], sl)
                    if prev is not None:
                        v_proj(j - 1, prev)
                    prev = xc
                v_proj(n_ch - 1, prev)

            # ---------------- Phase B+C fused ----------------
            # Global software pipeline over (chunk jq, head h, item): scores
            # matmuls + pair-batched exp run SKEWP items ahead of the AV /
            # denominator stage, and the out-projection of chunk jq-1 is
            # drip-fed into the stream so the PE never waits on ACT.
            entries = []
            head_last = {}
            for jq in range(n_j):
                rows = live[jq]
                items = _make_items(rows)
                for h in range(HPG):
                    n0 = 0
                    for k2, it in enumerate(items):
                        entries.append(dict(
                            jq=jq, h=h, item=it, n0=n0,
                            first=(k2 == 0), last=(k2 == len(items) - 1),
                            r=len(rows)))
                        n0 += len(it)
                    head_last[(jq, h)] = len(entries) - 1

            evac_at = {}
            for (jq, h), last in head_last.items():
                evac_at.setdefault(last + SKEWP + EVAC_DELAY, []).append((jq, h))
            c_release = {head_last[(jq, HPG - 1)] + SKEWP + EVAC_DELAY + 2: jq
                         for jq in range(n_j - 1)}

            with tc.tile_pool(name="rcp", bufs=3) as rcp, \
                 tc.tile_pool(name="sm", bufs=6) as smp, \
                 tc.tile_pool(name="pr", bufs=SKEWP + 3) as prp, \
                 tc.tile_pool(name="acc", bufs=8) as accp, \
                 tc.tile_pool(name="og", bufs=2) as ogp, \
                 tc.tile_pool(name="psS", bufs=2, space="PSUM") as psS, \
                 tc.tile_pool(name="psAt", bufs=2, space="PSUM") as psAt, \
                 tc.tile_pool(name="psC", bufs=2, space="PSUM") as psC:

                state = {}    # (jq,h) -> dict(at_ps, acc)
                rcs = {}
                c_queue = []
                c_po = {}     # current po tile for the active C dch chain
                c_og = {}     # st -> og tile

                def pat_mul(dst_ap, pidx):
                    if GP_PATTERNS:
                        nc.gpsimd.scalar_tensor_tensor(
                            dst_ap, dst_ap, 1.0, mblk_t[:, pidx, :],
                            MULT, MULT)
                    else:
                        nc.vector.tensor_tensor(
                            dst_ap, dst_ap, mblk_t[:, pidx, :], MULT)

                def emit_front(e):
                    jq, h, it = e['jq'], e['h'], e['item']
                    qb = jq * SQ
                    if e['first']:
                        state[(jq, h)] = dict(
                            acc=accp.tile([P, SQ], BF16, tag="acc", name="acc"))
                    st_ = state[(jq, h)]
                    if e['first']:
                        # row 0: single full-width exp writes the denominator
                        # accumulator directly
                        (i, c0, c1, pats) = it[0]
                        sc = psS.tile([P, 2 * SQ], F32, tag="sc", name="sc")
                        nc.tensor.matmul(
                            sc[:, c0:c1],
                            kh_sb[:, h, i * P:(i + 1) * P],
                            qh_sb[:, h, qb + c0:qb + c1],
                            start=True, stop=True)
                        acc = st_['acc']
                        nc.scalar.activation(
                            acc[:, c0:c1], sc[:, c0:c1],
                            mybir.ActivationFunctionType.Exp, scale=SCALE)
                        for (m, pidx) in pats:
                            pat_mul(acc[:, m * P:(m + 1) * P], pidx)
                        e['src'] = acc
                        e['offs'] = [0]
                        return
                    sc = psS.tile([P, 2 * SQ], F32, tag="sc", name="sc")
                    pr = prp.tile([P, 2 * SQ], BF16, tag="pr", name="pr")
                    for k2, (i, c0, c1, pats) in enumerate(it):
                        off = k2 * SQ
                        nc.tensor.matmul(
                            sc[:, off + c0:off + c1],
                            kh_sb[:, h, i * P:(i + 1) * P],
                            qh_sb[:, h, qb + c0:qb + c1],
                            start=True, stop=True)
                    # one exp per pair when contiguous; split when row B has
                    # a dead prefix so stale PSUM is never exp'd (no
                    # manufactured inf/nan in pr)
                    if len(it) == 2 and it[1][1] > 0:
                        for k2, (i, c0, c1, pats) in enumerate(it):
                            off = k2 * SQ
                            nc.scalar.activation(
                                pr[:, off + c0:off + c1],
                                sc[:, off + c0:off + c1],
                                mybir.ActivationFunctionType.Exp, scale=SCALE)
                    else:
                        c0a = it[0][1]
                        end = (len(it) - 1) * SQ + it[-1][2]
                        nc.scalar.activation(
                            pr[:, c0a:end], sc[:, c0a:end],
                            mybir.ActivationFunctionType.Exp, scale=SCALE)
                    for k2, (i, c0, c1, pats) in enumerate(it):
                        off = k2 * SQ
                        for (m, pidx) in pats:
                            pat_mul(pr[:, off + m * P:off + (m + 1) * P], pidx)
                    e['src'] = pr
                    e['offs'] = [k2 * SQ for k2 in range(len(it))]

                def emit_back(e):
                    jq, h, it = e['jq'], e['h'], e['item']
                    st_ = state[(jq, h)]
                    src = e['src']
                    acc = st_['acc']
                    for k2, (i, c0, c1, pats) in enumerate(it):
                        n = e['n0'] + k2
                        off = e['offs'][k2]
                        if n == 0:
                            st_['at_ps'] = psAt.tile([P, SQ], F32, tag="at", name="at")
                        at_ps = st_['at_ps']
                        nc.tensor.matmul(
                            at_ps[:, c0:c1],
                            v_sb[:, i, h * HD:(h + 1) * HD],
                            src[:, off + c0:off + c1],
                            start=(n == 0), stop=(n == e['r'] - 1),
                            skip_group_check=True)
                        if n > 0:
                            wide = (c1 - c0) > 256
                            if wide or not GP_NARROW_ADDS:
                                nc.vector.tensor_tensor(
                                    acc[:, c0:c1], acc[:, c0:c1],
                                    src[:, off + c0:off + c1], ADD)
                            else:
                                nc.gpsimd.scalar_tensor_tensor(
                                    acc[:, c0:c1], acc[:, c0:c1], 1.0,
                                    src[:, off + c0:off + c1], MULT, ADD)

                def emit_dn(jq, h):
                    acc = state[(jq, h)]['acc']
                    dn_ps = psS.tile([P, 2 * SQ], F32, tag="sc", name="dn_ps")
                    nc.tensor.matmul(dn_ps[0:1, 0:SQ], ones_b[:], acc[:],
                                     start=True, stop=True)
                    dn_sb = smp.tile([1, SQ], F32, tag="dnsb", name="dn_sb")
                    last = (jq == n_j - 1 and h == HPG - 1)
                    if last:
                        nc.vector.tensor_copy(dn_sb[:], dn_ps[0:1, 0:SQ])
                    else:
                        nc.scalar.activation(dn_sb[:], dn_ps[0:1, 0:SQ],
                                             mybir.ActivationFunctionType.Copy)
                    if last:
                        # tail chain gates the final out-projection: skip
                        # the fold hops, reciprocal directly on (1,512)
                        rc1 = smp.tile([1, SQ], F32, tag="rc1", name="rc1d")
                        nc.vector.reciprocal_approx_fast(rc1[:], dn_sb[:])
                    else:
                        # reciprocal cost scales with free-size per lane:
                        # fold the (1,512) row to (4,128) via DMA first
                        dn4 = smp.tile([SQ // P, P], F32, tag="dn4",
                                       name="dn4")
                        nc.gpsimd.dma_start(dn4[:], dn_sb[:])
                        rc4 = smp.tile([SQ // P, P], F32, tag="rc4",
                                       name="rc4")
                        nc.vector.reciprocal_approx_fast(rc4[:], dn4[:])
                        rc1 = smp.tile([1, SQ], F32, tag="rc1", name="rc1")
                        nc.gpsimd.dma_start(rc1[:], rc4[:])
                    # broadcast 1/dn to all partitions on GpSimd so the DVE
                    # evacuation TT has a legal SBUF second operand and the
                    # PE skips the ones-broadcast matmul
                    bc_sb = rcp.tile([P, SQ], F32, tag="rc", name="bc_sb")
                    nc.gpsimd.partition_broadcast(bc_sb[:], rc1[:])
                    rcs[(jq, h)] = bc_sb
                    if jq == 0:
                        # chunk-0 heads are only ~2us apart: evacuate the AV
                        # accumulator unnormalized NOW so psAt never starves;
                        # the normalization TT happens at the usual evac slot
                        at_ps = state[(jq, h)].pop('at_ps')
                        at_sb = accp.tile([P, SQ], BF16, tag="acc",
                                          name="at_sb")
                        nc.vector.tensor_copy(at_sb[:], at_ps[:])
                        state[(jq, h)]['at_sb'] = at_sb

                def emit_evac(jq, h):
                    bc_sb = rcs.pop((jq, h))
                    qsl = slice(jq * SQ, (jq + 1) * SQ)
                    if jq == 0:
                        at_sb = state[(jq, h)].pop('at_sb')
                        nc.vector.tensor_tensor(
                            attn_t[:, h, qsl], at_sb[:], bc_sb[:], MULT)
                    else:
                        at_ps = state[(jq, h)].pop('at_ps')
                        nc.vector.tensor_tensor(
                            attn_t[:, h, qsl], at_ps[:], bc_sb[:], MULT)

                def push_c_ops(jq, final=False):
                    for st2 in range(SQ // P):
                        st = jq * (SQ // P) + st2
                        for dch in range(nd):
                            for ct in range(HPG):
                                c_queue.append((st, dch, ct, final))

                def pop_c(quota):
                    npop = min(quota, len(c_queue))
                    for _ in range(npop):
                        (st, dch, ct, final) = c_queue.pop(0)
                        if ct == 0 and dch == 0:
                            c_og[st] = ogp.tile([P, d], BF16, tag="og", name="og")
                        if ct == 0:
                            c_po[st] = psC.tile([P, SQ], F32, tag="po", name="po")
                        po = c_po[st]
                        nc.tensor.matmul(
                            po[:], attn_t[:, ct, st * P:(st + 1) * P],
                            wo_t[:, ct, dch * SQ:(dch + 1) * SQ],
                            start=(ct == 0), stop=(ct == HPG - 1))
                        if ct != HPG - 1:
                            continue
                        og = c_og[st]
                        osl = slice(dch * SQ, (dch + 1) * SQ)
                        if dch % 4 == 3:
                            nc.scalar.activation(
                                og[:, osl], po[:],
                                mybir.ActivationFunctionType.Copy)
                        else:
                            nc.vector.tensor_copy(og[:, osl], po[:])
                        rsl = slice(st * P, (st + 1) * P)
                        if final:
                            # final chunk: write each dch immediately on a
                            # rotating queue so the tail DMA drains early
                            eng = (nc.sync, nc.gpsimd, nc.scalar,
                                   nc.sync)[(st + dch) % 4]
                            eng.dma_start(out[rsl, osl], og[:, osl])
                        elif dch % 2 == 1:
                            eng = nc.sync if (st + dch) % 4 == 1 else nc.gpsimd
                            eng.dma_start(
                                out[rsl, (dch - 1) * SQ:(dch + 1) * SQ],
                                og[:, (dch - 1) * SQ:(dch + 1) * SQ])

                total = len(entries) + SKEWP
                for idx in range(total):
                    if idx < len(entries):
                        emit_front(entries[idx])
                    bidx = idx - SKEWP
                    if bidx >= 0:
                        e = entries[bidx]
                        emit_back(e)
                        if e['last']:
                            emit_dn(e['jq'], e['h'])
                    for (jq, h) in evac_at.get(idx, []):
                        emit_evac(jq, h)
                    if idx in c_release:
                        push_c_ops(c_release[idx])
                    if len(c_queue) > C_RESERVE:
                        pop_c(min(C_QUOTA, len(c_queue) - C_RESERVE))

                # tail: held-back C ops cover the last rc chain, then the
                # final chunk's evacs and out-projection drain
                pop_c(C_RESERVE)
                for k in sorted(k for k in evac_at if k >= total):
                    for (jq, h) in evac_at[k]:
                        emit_evac(jq, h)
                push_c_ops(n_j - 1, final=True)
                pop_c(len(c_queue))

    nc.compile()
    return nc


def _prep_host(inputs):
    """Shard + transpose the full inputs into 8 per-core input maps."""
    x = np.asarray(inputs["x"], np.float32)
    wq = np.asarray(inputs["wq"], np.float32)
    wk = np.asarray(inputs["wk"], np.float32)
    wv = np.asarray(inputs["wv"], np.float32)
    wo = np.asarray(inputs["wo"], np.float32)
    cos = np.asarray(inputs["cos"], np.float32)
    sin = np.asarray(inputs["sin"], np.float32)
    mask = np.asarray(inputs["mask"], np.float32)
    start_p = int(inputs["start_p"])

    s = x.shape[1]
    cos_u = cos[start_p:start_p + s]          # (s, HD/2)
    sin_u = sin[start_p:start_p + s]

    # rotate-half channel permutation within each head: [evens, odds]
    perm = np.concatenate(
        [h * HD + np.concatenate([np.arange(0, HD, 2), np.arange(1, HD, 2)])
         for h in range(H)])

    cosP = np.ascontiguousarray(
        np.concatenate([cos_u.T, cos_u.T], axis=0)).astype(
            ml_dtypes.bfloat16)                              # (128, s)
    sinSw = np.ascontiguousarray(
        np.concatenate([sin_u.T, -sin_u.T], axis=0)).astype(
            ml_dtypes.bfloat16)                              # (128, s)

    live, pats = _classify_mask(mask)
    onesb = np.ones((P, 1), ml_dtypes.bfloat16)
    onesf = np.ones((1, P), np.float32)
    mblk = np.ascontiguousarray(pats.transpose(1, 0, 2)).astype(
        ml_dtypes.bfloat16)

    in_maps = []
    for b in range(B):
        xTp = _pre_x(np.ascontiguousarray(x[b].T))
        for g in range(GROUPS):
            rows = perm[g * C:(g + 1) * C]
            in_maps.append({
                "xT": xTp,
                "wqT": _pre_wqk(wq[rows, :].T),
                "wkT": _pre_wqk(wk[rows, :].T),
                "wvT": _pre_w(wv[g * C:(g + 1) * C, :].T),
                "woT": _pre_w(wo[:, g * C:(g + 1) * C].T),
                "cosP": cosP,
                "sinSw": sinSw,
                "mblk": mblk,
                "onesb": onesb,
                "onesf": onesf,
            })
    return in_maps, live, pats


def _run(inputs, trace=False):
    in_maps, live, pats = _prep_host(inputs)
    key = (pats.shape[0], _live_key(live))
    if key not in _PROGRAM_CACHE:
        _PROGRAM_CACHE[key] = _build(live, pats.shape[0])
    nc = _PROGRAM_CACHE[key]
    res = bass_utils.run_bass_kernel_spmd(
        nc, in_maps, core_ids=list(range(NCORES)), trace=trace)
    out = np.zeros((B, S, D), np.float32)
    for b in range(B):
        acc = res.results[b * GROUPS]["out"].astype(np.float32)
        for g in range(1, GROUPS):
            acc += res.results[b * GROUPS + g]["out"].astype(np.float32)
        out[b] = acc
    return out, res


def kernel(**inputs):
    out, _ = _run(inputs, trace=False)
    return out


# revision 26
# speedup vs baseline: 1.2167x; 1.0016x over previous
"""Trainium2 Bass kernel for nn_Attention_51307679318359.

Multi-head attention (B=2, S=2048, D=2048, H=16, HD=128) with RoPE and an
additive mask, sharded over 8 NeuronCores as (batch x head-group): each core
computes 1 batch and 4 heads (512 channels), producing a partial output that
the host sums over head-groups.

All matmul operands are bf16 (fast weight load, half the HBM traffic); PSUM
accumulation stays fp32, so accuracy is well inside the 2e-2 gate.  Q/K/V and
the attention output live entirely in SBUF -- no DRAM spills.

Per-core dataflow (activations transposed, channels on partitions):
  Warmup: ~4us of junk matmuls on a memset tile release the PE HAM clock gate
  (1.2 -> 2.4 GHz) before the first weight/x pieces land; startup DMAs lead
  with 128KB pieces in need-order on the sync/scalar queues (wq | x), with
  trig + wk + wo on gpsimd and wv split across sync/scalar behind chunk 1.
  A GpSimd PartitionBroadcast is primed under phase A (its first use pays a
  ~7us Q7 program load).
  Phase A: QT = wq' @ xT, KT = wk' @ xT with rotate-half permuted weights,
  RoPE applied on the PSUM output (ACT copy + DVE muls, ct-major after chunk
  0 so the DVE queue drains before phase B); V = xT' @ wv with ACT-copy
  evacuation.
  Phase B+C (fused): one global software pipeline over (sq-chunk, head,
  item).  Score tiles are matmul'd in pairs into a 2-bank PSUM tile so one
  ACT exp covers both (halving exp instruction overhead); exp(mask) patterns
  multiply only the diagonal sub-blocks; AV accumulates in PSUM while the
  denominator accumulates on the DVE in bf16; 1/denominator via
  reciprocal_approx_fast on a DMA-folded (4,128) row, partition-broadcast to
  all lanes on GpSimd, and folded into the PSUM->SBUF evacuation TT.  The
  out-projection of chunk jq-1 is drip-fed MM-by-MM into chunk jq's
  score/AV stream (with a reserved tail batch covering the last rc chain),
  so the PE never idles on exp latency and the old phase-C window
  disappears.  Output rows stream to HBM per dch-pair on rotating queues,
  per-dch for the final chunk.
"""

import math

import numpy as np
import ml_dtypes

import concourse.bass as bass
import concourse.mybir as mybir
import concourse.tile as tile
from concourse import bacc
from concourse import bass_utils

F32 = mybir.dt.float32
F32R = mybir.dt.float32r
BF16 = mybir.dt.bfloat16
ADD = mybir.AluOpType.add
MULT = mybir.AluOpType.mult

B, S, D = 2, 2048, 2048
H, HD = 16, 128
NCORES = 8
GROUPS = NCORES // B          # 4 head-groups
HPG = H // GROUPS             # 4 heads per group
C = HPG * HD                  # 512 per-core channels
P = 128
CH = 512                      # phase-A s-chunk width
SQ = 512                      # phase-B sq-chunk width
NKT = D // P                  # 16 k-tiles
NKB = 4                       # k-blocks
NKI = NKT // NKB              # k-tiles per block
NCT = C // P                  # 4 column tiles (= heads per group)
SCALE = 1.0 / math.sqrt(HD)
NEG_THRESH = -1e8             # "masked out" threshold

N_WARM = 40                   # junk warmup matmuls (N=64)
SKEWP = 2                     # item lookahead between front and back stages
EVAC_DELAY = 2                # items after a head's last back before bc+evac
C_QUOTA = 3                   # max C-ops popped per item step
C_RESERVE = 16                 # C-ops held back as tail filler
GP_PATTERNS = False            # pattern muls on GpSimd instead of DVE
GP_NARROW_ADDS = False         # narrow acc adds on GpSimd

_PROGRAM_CACHE = {}


def _pre_wqk(wT):
    """(d, c) -> [P, kb, ct, ki, 128] k-block-major contiguous bf16."""
    a = wT.reshape(NKB, NKI, P, NCT, P)              # [kb, ki, p, ct, col]
    return np.ascontiguousarray(a.transpose(2, 0, 3, 1, 4)).astype(
        ml_dtypes.bfloat16)


def _pre_w(wT):
    """(d, c) -> (128, d//128, c) partition-major contiguous bf16."""
    d, c = wT.shape
    return np.ascontiguousarray(
        wT.reshape(d // P, P, c).transpose(1, 0, 2)).astype(ml_dtypes.bfloat16)


def _pre_x(xT):
    """(d, s) -> (s//CH, 128, d//128, CH) chunk-major contiguous bf16."""
    d, s = xT.shape
    return np.ascontiguousarray(
        xT.reshape(d // P, P, s // CH, CH).transpose(2, 1, 0, 3)).astype(
            ml_dtypes.bfloat16)


def _classify_mask(mask):
    """Classify transposed-mask 128x128 sub-blocks per (sq-chunk jq, sk-tile i,
    sub-col m).  Returns (live, patterns):
      live[jq] = list of (i, c0, c1, [(m, pat_idx), ...]): per live sk tile,
        the live column range [c0, c1) within the chunk and the patterned
        128-col sub-blocks; the first entry always has c0 == 0 and c1 == SQ.
      patterns: (nblk, 128, 128) f32 holding exp(maskT sub-block).
    """
    maskT = np.ascontiguousarray(mask.T)
    s = mask.shape[0]
    n_j = s // SQ
    n_i = s // P
    n_m = SQ // P
    patterns = []
    pat_idx = {}

    def blk_of(jq, i, m):
        return maskT[i * P:(i + 1) * P, jq * SQ + m * P:jq * SQ + (m + 1) * P]

    def add_pattern(blk):
        key = blk.tobytes()
        if key not in pat_idx:
            pat_idx[key] = len(patterns)
            with np.errstate(over='ignore'):
                patterns.append(np.exp(blk.astype(np.float64)).astype(np.float32))
        return pat_idx[key]

    cls = {}
    for jq in range(n_j):
        for i in range(n_i):
            for m in range(n_m):
                blk = blk_of(jq, i, m)
                if np.all(blk == 0.0):
                    cls[(jq, i, m)] = 'plain'
                elif np.all(blk <= NEG_THRESH):
                    cls[(jq, i, m)] = 'skip'
                else:
                    cls[(jq, i, m)] = add_pattern(blk)

    def full_rows(jq):
        rows = []
        for i in range(n_i):
            pats = []
            for m in range(n_m):
                c = cls[(jq, i, m)]
                if c == 'skip':
                    pats.append((m, add_pattern(blk_of(jq, i, m))))
                elif isinstance(c, int):
                    pats.append((m, c))
            rows.append((i, 0, SQ, pats))
        return rows

    live = {}
    for jq in range(n_j):
        rows = []
        for i in range(n_i):
            ms = [m for m in range(n_m) if cls[(jq, i, m)] != 'skip']
            if not ms:
                continue
            m0, m1 = min(ms), max(ms)
            pats = []
            for m in range(m0, m1 + 1):
                c = cls[(jq, i, m)]
                if c == 'skip':
                    # interior hole: apply its (all-zero) pattern explicitly
                    pats.append((m, add_pattern(blk_of(jq, i, m))))
                elif isinstance(c, int):
                    pats.append((m, c))
            rows.append((i, m0 * P, (m1 + 1) * P, pats))
        # PSUM accumulation needs a full-width first tile, and softmax needs
        # every column covered; fall back to no narrowing for odd masks.
        cov = np.zeros(SQ, bool)
        for (_, c0, c1, _) in rows:
            cov[c0:c1] = True
        if not rows or not cov.all():
            rows = full_rows(jq)
        elif rows[0][1] != 0 or rows[0][2] != SQ:
            # move a full-width tile to the front if one exists, else fall back
            fw = [r for r in rows if r[1] == 0 and r[2] == SQ]
            if fw:
                rows.remove(fw[0])
                rows.insert(0, fw[0])
            else:
                rows = full_rows(jq)
        live[jq] = rows
    pats = np.stack(patterns, 0).astype(np.float32) if patterns else \
        np.zeros((1, P, P), np.float32)
    return live, pats


def _live_key(live):
    return tuple(sorted(
        (jq, tuple((i, c0, c1, tuple(p)) for (i, c0, c1, p) in rows))
        for jq, rows in live.items()))


def _make_items(rows):
    """Group a head-chunk's rows into exp-batchable items.

    Row 0 (full width) stays single: its exp writes the denominator
    accumulator tile directly.  Remaining rows are paired so one ACT exp
    covers a 2-bank PSUM tile; a trailing odd row stays single."""
    items = [[rows[0]]]
    k = 1
    while k < len(rows):
        if k + 1 < len(rows):
            items.append([rows[k], rows[k + 1]])
            k += 2
        else:
            items.append([rows[k]])
            k += 1
    return items


def _build(live, nblk, s=S, d=D):
    """Build + compile the per-core SPMD program."""
    n_j = s // SQ
    n_i = s // P
    n_ch = s // CH
    nd = d // SQ

    nc = bacc.Bacc("TRN2", target_bir_lowering=False, debug=False)
    xT = nc.dram_tensor("xT", (n_ch, P, NKT, CH), BF16, kind="ExternalInput")
    wqT = nc.dram_tensor("wqT", (P, NKB, NCT, NKI, P), BF16,
                         kind="ExternalInput")
    wkT = nc.dram_tensor("wkT", (P, NKB, NCT, NKI, P), BF16,
                         kind="ExternalInput")
    wvT = nc.dram_tensor("wvT", (P, NKT, C), BF16, kind="ExternalInput")
    woT = nc.dram_tensor("woT", (P, HPG, d), BF16, kind="ExternalInput")
    cosP = nc.dram_tensor("cosP", (HD, s), BF16, kind="ExternalInput")
    sinSw = nc.dram_tensor("sinSw", (HD, s), BF16, kind="ExternalInput")
    mblk = nc.dram_tensor("mblk", (P, nblk, P), BF16, kind="ExternalInput")
    onesb = nc.dram_tensor("onesb", (P, 1), BF16, kind="ExternalInput")
    onesf = nc.dram_tensor("onesf", (1, P), F32, kind="ExternalInput")
    out = nc.dram_tensor("out", (s, d), BF16, kind="ExternalOutput")

    with tile.TileContext(nc) as tc:
        with tc.tile_pool(name="const", bufs=1) as const:
            ones_b = const.tile([P, 1], BF16)
            ones_f = const.tile([1, P], F32R)
            mblk_t = const.tile([P, nblk, P], BF16)

            qh_sb = const.tile([P, HPG, s], BF16)
            kh_sb = const.tile([P, HPG, s], BF16)
            v_sb = const.tile([P, n_i, C], BF16)
            attn_t = const.tile([P, HPG, s], BF16)
            wo_t = const.tile([P, HPG, d], BF16)

            # ---------------- Phase A: QKV projections + RoPE ----------------
            with tc.tile_pool(name="wres", bufs=1) as wres, \
                 tc.tile_pool(name="xc", bufs=3) as xcp, \
                 tc.tile_pool(name="trig", bufs=1) as trig, \
                 tc.tile_pool(name="ptmp", bufs=2) as ptmp, \
                 tc.tile_pool(name="psA", bufs=6, space="PSUM") as psA, \
                 tc.tile_pool(name="psV", bufs=2, space="PSUM") as psV:
                wq_t = wres.tile([P, NKB, NCT, NKI, P], BF16)
                wk_t = wres.tile([P, NKB, NCT, NKI, P], BF16)
                wv_t = wres.tile([P, NKT, C], BF16)
                junk = wres.tile([P, 64], BF16)
                dum1 = wres.tile([1, P], F32)
                dum2 = wres.tile([P, P], F32)
                cos_t = trig.tile([P, s], BF16)
                sin_t = trig.tile([P, s], BF16)

                # PE warmup: junk matmuls on a memset tile release the HAM
                # clock gate (1.2 -> 2.4 GHz) before the first real matmul
                nc.gpsimd.memset(junk[:], 0)
                nc.gpsimd.memset(dum1[:], 1)
                psW = psA.tile([P, CH], F32, tag="ps_qk", name="ps_warm")
                for _ in range(N_WARM):
                    nc.tensor.matmul(psW[0:64, 0:64], junk[:], junk[:],
                                     start=True, stop=True)

                # startup (3 DMA queues: sync, scalar, gpsimd).  The first
                # matmul group needs only the wq[kb0,ct0] piece and xc0
                # k-tile 0, so sync/scalar lead with 128KB pieces in need
                # order; wk + trig ride gpsimd (K starts ~14us after Q).
                for ct in range(NCT):
                    nc.sync.dma_start(wq_t[:, 0, ct], wqT[:, 0, ct])
                for kb in range(1, NKB):
                    nc.sync.dma_start(wq_t[:, kb], wqT[:, kb])
                xc0 = xcp.tile([P, NKT, CH], BF16, tag="xc")
                for k in range(NKI):
                    nc.scalar.dma_start(xc0[:, k:k + 1, :], xT[0, :, k:k + 1, :])
                for kb in range(1, NKB):
                    nc.scalar.dma_start(xc0[:, kb * NKI:(kb + 1) * NKI, :],
                                        xT[0, :, kb * NKI:(kb + 1) * NKI, :])
                nc.gpsimd.dma_start(cos_t[:], cosP[:])
                nc.gpsimd.dma_start(sin_t[:], sinSw[:])
                nc.gpsimd.dma_start(mblk_t[:], mblk[:])
                nc.gpsimd.dma_start(ones_b[:], onesb[:])
                nc.gpsimd.dma_start(ones_f[:], onesf[:].bitcast(F32R))
                for ct in range(NCT):
                    nc.gpsimd.dma_start(wk_t[:, 0, ct], wkT[:, 0, ct])
                for kb in range(1, NKB):
                    nc.gpsimd.dma_start(wk_t[:, kb], wkT[:, kb])
                nc.gpsimd.dma_start(wo_t[:], woT[:])
                # prime the GpSimd custom-op program (PartitionBroadcast's
                # first use pays a ~7us Q7 library load; do it under phase A)
                nc.gpsimd.partition_broadcast(dum2[:], dum1[:])

                def rope_store(ps, dst, sl):
                    # rotate-half RoPE on a finished PSUM tile -> bf16 SBUF
                    pc = ptmp.tile([P, CH], F32, tag="pc")
                    nc.scalar.activation(
                        pc[:], ps[:], mybir.ActivationFunctionType.Copy)
                    t1 = ptmp.tile([P, CH], F32, tag="t1")
                    nc.vector.tensor_tensor(t1[:], pc[:], cos_t[:, sl], MULT)
                    t2 = ptmp.tile([P, CH], F32, tag="t2")
                    nc.vector.tensor_tensor(
                        t2[0:64, :], pc[64:128, :], sin_t[64:128, sl], MULT)
                    nc.vector.tensor_tensor(
                        t2[64:128, :], pc[0:64, :], sin_t[0:64, sl], MULT)
                    nc.vector.tensor_tensor(dst, t1[:], t2[:], ADD)

                def v_proj(jv, xcv):
                    for st2 in range(CH // P):
                        st = (jv * CH) // P + st2
                        psv = psV.tile([P, C], F32, tag="ps_v")
                        for k in range(NKT):
                            nc.tensor.matmul(
                                psv[:], xcv[:, k, st2 * P:(st2 + 1) * P],
                                wv_t[:, k, :],
                                start=(k == 0), stop=(k == NKT - 1))
                        # ACT copy keeps the DVE queue clear for phase B
                        nc.scalar.activation(
                            v_sb[:, st, :], psv[:],
                            mybir.ActivationFunctionType.Copy)

                prev = None
                nxt = None
                for j in range(n_ch):
                    sl = slice(j * CH, (j + 1) * CH)
                    xc = xc0 if j == 0 else nxt
                    # prefetch the next chunk a full iteration ahead, split
                    # across the scalar and sync queues so neither starves
                    if j + 1 < n_ch:
                        nxt = xcp.tile([P, NKT, CH], BF16, tag="xc")
                        for kb in range(NKB):
                            eng = nc.scalar if kb % 2 == 0 else nc.sync
                            eng.dma_start(
                                nxt[:, kb * NKI:(kb + 1) * NKI, :],
                                xT[j + 1, :, kb * NKI:(kb + 1) * NKI, :])
                    if j == 0:
                        # wv rides behind chunk 1's prefetch on both queues
                        nc.scalar.dma_start(wv_t[:, 0:NKT // 2, :],
                                            wvT[:, 0:NKT // 2, :])
                        nc.sync.dma_start(wv_t[:, NKT // 2:, :],
                                          wvT[:, NKT // 2:, :])
                    for (wt, dst) in ((wq_t, qh_sb), (wk_t, kh_sb)):
                        if j == 0:
                            # kb-major: consumes startup DMA pieces in
                            # arrival order
                            pss = [psA.tile([P, CH], F32, tag="ps_qk",
                                            name=f"ps_qk{ct}")
                                   for ct in range(NCT)]
                            for kb in range(NKB):
                                for ct in range(NCT):
                                    for ki in range(NKI):
                                        nc.tensor.matmul(
                                            pss[ct][:], wt[:, kb, ct, ki, :],
                                            xc[:, kb * NKI + ki, :],
                                            start=(kb == 0 and ki == 0),
                                            stop=(kb == NKB - 1 and
                                                  ki == NKI - 1),
                                            skip_group_check=True)
                            for ct in range(NCT):
                                rope_store(pss[ct], dst[:, ct, sl], sl)
                        else:
                            # ct-major: each rope starts right after its own
                            # 16-MM chain, so the DVE queue drains during the
                            # chunk instead of piling up at the A->B boundary
                            for ct in range(NCT):
                                psc = psA.tile([P, CH], F32, tag="ps_qk",
                                               name=f"ps_qk{ct}")
                                for kb in range(NKB):
                                    for ki in range(NKI):
                                        nc.tensor.matmul(
                                            psc[:], wt[:, kb, ct, ki, :],
                                            xc[:, kb * NKI + ki, :],
                                            start=(kb == 0 and ki == 0),
                                            stop=(kb == NKB - 1 and
                                                  ki == NKI - 1),
                                            skip_group_check=True)
                                rope_store(psc, dst[:, ct, sl], sl)
                    if prev is not None:
                        v_proj(j - 1, prev)
                    prev = xc
                v_proj(n_ch - 1, prev)

            # ---------------- Phase B+C fused ----------------
            # Global software pipeline over (chunk jq, head h, item): scores
            # matmuls + pair-batched exp run SKEWP items ahead of the AV /
            # denominator stage, and the out-projection of chunk jq-1 is
            # drip-fed into the stream so the PE never waits on ACT.
            entries = []
            head_last = {}
            for jq in range(n_j):
                rows = live[jq]
                items = _make_items(rows)
                for h in range(HPG):
                    n0 = 0
                    for k2, it in enumerate(items):
                        entries.append(dict(
                            jq=jq, h=h, item=it, n0=n0,
                            first=(k2 == 0), last=(k2 == len(items) - 1),
                            r=len(rows)))
                        n0 += len(it)
                    head_last[(jq, h)] = len(entries) - 1

            evac_at = {}
            for (jq, h), last in head_last.items():
                evac_at.setdefault(last + SKEWP + EVAC_DELAY, []).append((jq, h))
            c_release = {head_last[(jq, HPG - 1)] + SKEWP + EVAC_DELAY + 2: jq
                         for jq in range(n_j - 1)}

            with tc.tile_pool(name="rcp", bufs=3) as rcp, \
                 tc.tile_pool(name="sm", bufs=6) as smp, \
                 tc.tile_pool(name="pr", bufs=SKEWP + 3) as prp, \
                 tc.tile_pool(name="acc", bufs=8) as accp, \
                 tc.tile_pool(name="og", bufs=2) as ogp, \
                 tc.tile_pool(name="psS", bufs=2, space="PSUM") as psS, \
                 tc.tile_pool(name="psAt", bufs=2, space="PSUM") as psAt, \
                 tc.tile_pool(name="psC", bufs=2, space="PSUM") as psC:

                state = {}    # (jq,h) -> dict(at_ps, acc)
                rcs = {}
                c_queue = []
                c_po = {}     # current po tile for the active C dch chain
                c_og = {}     # st -> og tile

                def pat_mul(dst_ap, pidx):
                    if GP_PATTERNS:
                        nc.gpsimd.scalar_tensor_tensor(
                            dst_ap, dst_ap, 1.0, mblk_t[:, pidx, :],
                            MULT, MULT)
                    else:
                        nc.vector.tensor_tensor(
                            dst_ap, dst_ap, mblk_t[:, pidx, :], MULT)

                def emit_front(e):
                    jq, h, it = e['jq'], e['h'], e['item']
                    qb = jq * SQ
                    if e['first']:
                        state[(jq, h)] = dict(
                            acc=accp.tile([P, SQ], BF16, tag="acc", name="acc"))
                    st_ = state[(jq, h)]
                    if e['first']:
                        # row 0: single full-width exp writes the denominator
                        # accumulator directly
                        (i, c0, c1, pats) = it[0]
                        sc = psS.tile([P, 2 * SQ], F32, tag="sc", name="sc")
                        nc.tensor.matmul(
                            sc[:, c0:c1],
                            kh_sb[:, h, i * P:(i + 1) * P],
                            qh_sb[:, h, qb + c0:qb + c1],
                            start=True, stop=True)
                        acc = st_['acc']
                        nc.scalar.activation(
                            acc[:, c0:c1], sc[:, c0:c1],
                            mybir.ActivationFunctionType.Exp, scale=SCALE)
                        for (m, pidx) in pats:
                            pat_mul(acc[:, m * P:(m + 1) * P], pidx)
                        e['src'] = acc
                        e['offs'] = [0]
                        return
                    sc = psS.tile([P, 2 * SQ], F32, tag="sc", name="sc")
                    pr = prp.tile([P, 2 * SQ], BF16, tag="pr", name="pr")
                    for k2, (i, c0, c1, pats) in enumerate(it):
                        off = k2 * SQ
                        nc.tensor.matmul(
                            sc[:, off + c0:off + c1],
                            kh_sb[:, h, i * P:(i + 1) * P],
                            qh_sb[:, h, qb + c0:qb + c1],
                            start=True, stop=True)
                    # one exp per pair when contiguous; split when row B has
                    # a dead prefix so stale PSUM is never exp'd (no
                    # manufactured inf/nan in pr)
                    if len(it) == 2 and it[1][1] > 0:
                        for k2, (i, c0, c1, pats) in enumerate(it):
                            off = k2 * SQ
                            nc.scalar.activation(
                                pr[:, off + c0:off + c1],
                                sc[:, off + c0:off + c1],
                                mybir.ActivationFunctionType.Exp, scale=SCALE)
                    else:
                        c0a = it[0][1]
                        end = (len(it) - 1) * SQ + it[-1][2]
                        nc.scalar.activation(
                            pr[:, c0a:end], sc[:, c0a:end],
                            mybir.ActivationFunctionType.Exp, scale=SCALE)
                    for k2, (i, c0, c1, pats) in enumerate(it):
                        off = k2 * SQ
                        for (m, pidx) in pats:
                            pat_mul(pr[:, off + m * P:off + (m + 1) * P], pidx)
                    e['src'] = pr
                    e['offs'] = [k2 * SQ for k2 in range(len(it))]

                def emit_back(e):
                    jq, h, it = e['jq'], e['h'], e['item']
                    st_ = state[(jq, h)]
                    src = e['src']
                    acc = st_['acc']
                    for k2, (i, c0, c1, pats) in enumerate(it):
                        n = e['n0'] + k2
                        off = e['offs'][k2]
                        if n == 0:
                            st_['at_ps'] = psAt.tile([P, SQ], F32, tag="at", name="at")
                        at_ps = st_['at_ps']
                        nc.tensor.matmul(
                            at_ps[:, c0:c1],
                            v_sb[:, i, h * HD:(h + 1) * HD],
                            src[:, off + c0:off + c1],
                            start=(n == 0), stop=(n == e['r'] - 1),
                            skip_group_check=True)
                        if n > 0:
                            wide = (c1 - c0) > 256
                            if wide or not GP_NARROW_ADDS:
                                nc.vector.tensor_tensor(
                                    acc[:, c0:c1], acc[:, c0:c1],
                                    src[:, off + c0:off + c1], ADD)
                            else:
                                nc.gpsimd.scalar_tensor_tensor(
                                    acc[:, c0:c1], acc[:, c0:c1], 1.0,
                                    src[:, off + c0:off + c1], MULT, ADD)

                def emit_dn(jq, h):
                    acc = state[(jq, h)]['acc']
                    dn_ps = psS.tile([P, 2 * SQ], F32, tag="sc", name="dn_ps")
                    nc.tensor.matmul(dn_ps[0:1, 0:SQ], ones_b[:], acc[:],
                                     start=True, stop=True)
                    dn_sb = smp.tile([1, SQ], F32, tag="dnsb", name="dn_sb")
                    last = (jq == n_j - 1 and h == HPG - 1)
                    if last:
                        nc.vector.tensor_copy(dn_sb[:], dn_ps[0:1, 0:SQ])
                    else:
                        nc.scalar.activation(dn_sb[:], dn_ps[0:1, 0:SQ],
                                             mybir.ActivationFunctionType.Copy)
                    if last:
                        # tail chain gates the final out-projection: skip
                        # the fold hops, reciprocal directly on (1,512)
                        rc1 = smp.tile([1, SQ], F32, tag="rc1", name="rc1d")
                        nc.vector.reciprocal_approx_fast(rc1[:], dn_sb[:])
                    else:
                        # reciprocal cost scales with free-size per lane:
                        # fold the (1,512) row to (4,128) via DMA first
                        dn4 = smp.tile([SQ // P, P], F32, tag="dn4",
                                       name="dn4")
                        nc.gpsimd.dma_start(dn4[:], dn_sb[:])
                        rc4 = smp.tile([SQ // P, P], F32, tag="rc4",
                                       name="rc4")
                        nc.vector.reciprocal_approx_fast(rc4[:], dn4[:])
                        rc1 = smp.tile([1, SQ], F32, tag="rc1", name="rc1")
                        nc.gpsimd.dma_start(rc1[:], rc4[:])
                    # broadcast 1/dn to all partitions on GpSimd so the DVE
                    # evacuation TT has a legal SBUF second operand and the
                    # PE skips the ones-broadcast matmul
                    bc_sb = rcp.tile([P, SQ], F32, tag="rc", name="bc_sb")
                    nc.gpsimd.partition_broadcast(bc_sb[:], rc1[:])
                    rcs[(jq, h)] = bc_sb
                    if jq == 0:
                        # chunk-0 heads are only ~2us apart: evacuate the AV
                        # accumulator unnormalized NOW so psAt never starves;
                        # the normalization TT happens at the usual evac slot
                        at_ps = state[(jq, h)].pop('at_ps')
                        at_sb = accp.tile([P, SQ], BF16, tag="acc",
                                          name="at_sb")
                        nc.vector.tensor_copy(at_sb[:], at_ps[:])
                        state[(jq, h)]['at_sb'] = at_sb

                def emit_evac(jq, h):
                    bc_sb = rcs.pop((jq, h))
                    qsl = slice(jq * SQ, (jq + 1) * SQ)
                    if jq == 0:
                        at_sb = state[(jq, h)].pop('at_sb')
                        nc.vector.tensor_tensor(
                            attn_t[:, h, qsl], at_sb[:], bc_sb[:], MULT)
                    else:
                        at_ps = state[(jq, h)].pop('at_ps')
                        nc.vector.tensor_tensor(
                            attn_t[:, h, qsl], at_ps[:], bc_sb[:], MULT)

                def push_c_ops(jq, final=False):
                    for st2 in range(SQ // P):
                        st = jq * (SQ // P) + st2
                        for dch in range(nd):
                            for ct in range(HPG):
                                c_queue.append((st, dch, ct, final))

                def pop_c(quota, act_evac=False):
                    npop = min(quota, len(c_queue))
                    for _ in range(npop):
                        (st, dch, ct, final) = c_queue.pop(0)
                        if ct == 0 and dch == 0:
                            c_og[st] = ogp.tile([P, d], BF16, tag="og", name="og")
                        if ct == 0:
                            c_po[st] = psC.tile([P, SQ], F32, tag="po", name="po")
                        po = c_po[st]
                        nc.tensor.matmul(
                            po[:], attn_t[:, ct, st * P:(st + 1) * P],
                            wo_t[:, ct, dch * SQ:(dch + 1) * SQ],
                            start=(ct == 0), stop=(ct == HPG - 1))
                        if ct != HPG - 1:
                            continue
                        og = c_og[st]
                        osl = slice(dch * SQ, (dch + 1) * SQ)
                        if act_evac or dch % 4 == 3:
                            nc.scalar.activation(
                                og[:, osl], po[:],
                                mybir.ActivationFunctionType.Copy)
                        else:
                            nc.vector.tensor_copy(og[:, osl], po[:])
                        rsl = slice(st * P, (st + 1) * P)
                        if final:
                            # final chunk: write each dch immediately on a
                            # rotating queue so the tail DMA drains early
                            eng = (nc.sync, nc.gpsimd, nc.scalar,
                                   nc.sync)[(st + dch) % 4]
                            eng.dma_start(out[rsl, osl], og[:, osl])
                        elif dch % 2 == 1:
                            eng = nc.sync if (st + dch) % 4 == 1 else nc.gpsimd
                            eng.dma_start(
                                out[rsl, (dch - 1) * SQ:(dch + 1) * SQ],
                                og[:, (dch - 1) * SQ:(dch + 1) * SQ])

                total = len(entries) + SKEWP
                for idx in range(total):
                    if idx < len(entries):
                        emit_front(entries[idx])
                    bidx = idx - SKEWP
                    if bidx >= 0:
                        e = entries[bidx]
                        emit_back(e)
                        if e['last']:
                            emit_dn(e['jq'], e['h'])
                    for (jq, h) in evac_at.get(idx, []):
                        emit_evac(jq, h)
                    if idx in c_release:
                        push_c_ops(c_release[idx])
                    if len(c_queue) > C_RESERVE:
                        pop_c(min(C_QUOTA, len(c_queue) - C_RESERVE))

                # tail: held-back C ops cover the last rc chain (their og
                # evacs ride ACT so the DVE-side chain is unobstructed)
                pop_c(C_RESERVE, act_evac=True)
                for k in sorted(k for k in evac_at if k >= total):
                    for (jq, h) in evac_at[k]:
                        emit_evac(jq, h)
                push_c_ops(n_j - 1, final=True)
                pop_c(len(c_queue))

    nc.compile()
    return nc


def _prep_host(inputs):
    """Shard + transpose the full inputs into 8 per-core input maps."""
    x = np.asarray(inputs["x"], np.float32)
    wq = np.asarray(inputs["wq"], np.float32)
    wk = np.asarray(inputs["wk"], np.float32)
    wv = np.asarray(inputs["wv"], np.float32)
    wo = np.asarray(inputs["wo"], np.float32)
    cos = np.asarray(inputs["cos"], np.float32)
    sin = np.asarray(inputs["sin"], np.float32)
    mask = np.asarray(inputs["mask"], np.float32)
    start_p = int(inputs["start_p"])

    s = x.shape[1]
    cos_u = cos[start_p:start_p + s]          # (s, HD/2)
    sin_u = sin[start_p:start_p + s]

    # rotate-half channel permutation within each head: [evens, odds]
    perm = np.concatenate(
        [h * HD + np.concatenate([np.arange(0, HD, 2), np.arange(1, HD, 2)])
         for h in range(H)])

    cosP = np.ascontiguousarray(
        np.concatenate([cos_u.T, cos_u.T], axis=0)).astype(
            ml_dtypes.bfloat16)                              # (128, s)
    sinSw = np.ascontiguousarray(
        np.concatenate([sin_u.T, -sin_u.T], axis=0)).astype(
            ml_dtypes.bfloat16)                              # (128, s)

    live, pats = _classify_mask(mask)
    onesb = np.ones((P, 1), ml_dtypes.bfloat16)
    onesf = np.ones((1, P), np.float32)
    mblk = np.ascontiguousarray(pats.transpose(1, 0, 2)).astype(
        ml_dtypes.bfloat16)

    in_maps = []
    for b in range(B):
        xTp = _pre_x(np.ascontiguousarray(x[b].T))
        for g in range(GROUPS):
            rows = perm[g * C:(g + 1) * C]
            in_maps.append({
                "xT": xTp,
                "wqT": _pre_wqk(wq[rows, :].T),
                "wkT": _pre_wqk(wk[rows, :].T),
                "wvT": _pre_w(wv[g * C:(g + 1) * C, :].T),
                "woT": _pre_w(wo[:, g * C:(g + 1) * C].T),
                "cosP": cosP,
                "sinSw": sinSw,
                "mblk": mblk,
                "onesb": onesb,
                "onesf": onesf,
            })
    return in_maps, live, pats


def _run(inputs, trace=False):
    in_maps, live, pats = _prep_host(inputs)
    key = (pats.shape[0], _live_key(live))
    if key not in _PROGRAM_CACHE:
        _PROGRAM_CACHE[key] = _build(live, pats.shape[0])
    nc = _PROGRAM_CACHE[key]
    res = bass_utils.run_bass_kernel_spmd(
        nc, in_maps, core_ids=list(range(NCORES)), trace=trace)
    out = np.zeros((B, S, D), np.float32)
    for b in range(B):
        acc = res.results[b * GROUPS]["out"].astype(np.float32)
        for g in range(1, GROUPS):
            acc += res.results[b * GROUPS + g]["out"].astype(np.float32)
        out[b] = acc
    return out, res


def kernel(**inputs):
    out, _ = _run(inputs, trace=False)
    return out
